# revision 1
# baseline (speedup 1.0000x reference)
"""CNLinkPredictor Trainium2 kernel.

Edge-sharded across 8 NeuronCores (1024 target edges each); x, adj, and the
MLP weights are replicated. Per core:
  A) h = x + MLP(x) computed in transposed layout: the host supplies xT, so
     stage A is matmul-only on PE (bf16, N=512 moving), fused bias+ReLU on
     the scalar engine, residual on DVE, then xbar DMA-transposes write h
     back to natural layout (bf16, (half, ktile, c) column order so every
     transpose destination is a contiguous per-partition span).
  B) per 128-edge block and k-half: indirect-DMA gather of the two adjacency
     rows per edge (fp8 - exact for a 0/1 adjacency - one row per SBUF
     partition), DVE multiply -> cn (bf16, still exact), one xbar
     DMA-transpose, then 32 matmuls accumulating cnT @ h into PSUM.
  C) edge MLPs in transposed layout (bf16, N=512 over 4-block groups), xbar
     transposes for xcn and xi*xj, final [1, 1024] output row.

Emission is software-pipelined (A first half, B k-half 0, A second half,
B k-half 1, C, ...) so the FIFO engine queues never head-of-line block on
data that is not ready yet.

Hardware pitfalls this kernel works around:
  - This walrus build accepts at most ONE sync-wait per instruction
    (_apply_tile_patch splits the Tile tail drain; _split_multi_waits hoists
    extra waits onto same-engine NoOps).
  - Concurrent 4-byte DMA traffic corrupts in-flight 2-byte xbar
    DMA-transposes, so every steady-state transfer is <= 2 bytes/element
    (fp8 adjacency, bf16 everything else); the few f32/int32 loads happen
    up front and the single f32 store happens after the last transpose.
  - xbar transposes into non-contiguous destinations produce wrong data;
    all transpose targets collapse to contiguous 2-D access patterns.
"""

import numpy as np
import ml_dtypes

N = 8192
C = 256
E = 8192
NCORES = 8
EL = E // NCORES          # edges per core
P = 128
NB = EL // P              # edge blocks per core
KH = 2                    # k halves for adjacency gather
KC = N // KH              # columns per half
NKT = N // P              # 64 k tiles
AGRP = 512                # stage-A node group
CGRP = 4                  # stage-C blocks per group (512 edges)

_CACHE = {}
TRACE = False
LAST_RESULT = None
DEBUG_DUMPS = False


def _apply_tile_patch():
    """Split the Tile tail-drain's multi-sem wait onto individual SP nops."""
    from concourse.tile import TileContext
    from concourse.vector_clock import ScopedClock

    if getattr(TileContext, "_drain_patched", False):
        return

    def _patched(self, tick_clock, wait_clock):
        nc = self.nc
        collector = nc.sync.nop()
        wait_clock.add_sem_waits(
            collector.ins, ScopedClock({None: tick_clock.global_clock})
        )
        si = collector.ins.sync_info
        waits = list(si.on_wait) if si is not None and si.on_wait else []
        if si is not None and len(waits) > 1:
            name_to_handle = {h.name: h for h in self.sems.allocated().values()}
            si.on_wait = [waits[0]]
            for w in waits[1:]:
                op = {
                    "sem-ge-imm": "sem-ge",
                    "sem-eq-imm": "sem-eq",
                    "sem-le-imm": "sem-le",
                }.get(str(w.wait_mode), "sem-ge")
                nc.sync.nop().wait_op(name_to_handle[w.ant_name], w.wait_value, op)
        nc.sync.drain()
        nc.all_engine_barrier()
        assert self.sems is not None
        popped = nc._tile_sem_poison_stack.pop()
        assert popped is self._sem_poison
        nc.clear_and_free_semaphores(list(self.sems.allocated().values()))
        nc.all_engine_barrier()

    TileContext._drain_and_barrier = _patched
    TileContext._drain_patched = True


def _split_multi_waits(nc):
    """Hoist extra sync-waits onto same-engine NoOps (sequential waits ==
    ANDed waits); this walrus build allows one wait per instruction."""
    import concourse.mybir as mybir

    cnt = 0
    for fn in nc.m.functions:
        for bb in fn.blocks:
            out = []
            for inst in bb.instructions:
                si = getattr(inst, "sync_info", None)
                waits = list(si.on_wait) if si is not None and si.on_wait else []
                if len(waits) > 1:
                    for w in waits[:-1]:
                        nop = mybir.InstNoOp(name=f"ws-{cnt}", ins=[], outs=[])
                        cnt += 1
                        nop.engine = inst.engine
                        nop.sync_info = mybir.SyncInfo(on_wait=[w], on_update=[])
                        out.append(nop)
                    si.on_wait = [waits[-1]]
                out.append(inst)
            bb.instructions = out
    return nc


def _build(split_waits=True):
    import concourse.bass as bass
    import concourse.mybir as mybir
    from concourse.tile import TileContext

    _apply_tile_patch()

    f32 = mybir.dt.float32
    f32r = mybir.dt.float32r
    bf16 = mybir.dt.bfloat16
    fp8 = mybir.dt.float8e4
    i32 = mybir.dt.int32
    Relu = mybir.ActivationFunctionType.Relu
    Ident = mybir.ActivationFunctionType.Identity
    MUL = mybir.AluOpType.mult
    ADD = mybir.AluOpType.add

    nc = bass.Bass(num_swdge_queues=4)

    xT_d = nc.dram_tensor("xT", [C, N], bf16, kind="ExternalInput")
    x_d = nc.dram_tensor("x", [N, C], bf16, kind="ExternalInput")
    adj_d = nc.dram_tensor("adj", [N, N], fp8, kind="ExternalInput")
    idx_d = nc.dram_tensor("idx", [2, EL], i32, kind="ExternalInput")
    # all matmul weights in bf16 (2-byte rule; see module docstring)
    wA = {n: nc.dram_tensor(n, [C, C], bf16, kind="ExternalInput")
          for n in ("xlin_w1", "xlin_w2")}
    wC = {n: nc.dram_tensor(n, [C, C], bf16, kind="ExternalInput")
          for n in ("xcn_w1", "xcn_w2", "xij_w", "lin_w1")}
    lin_w2_d = nc.dram_tensor("lin_w2", [C, 1], bf16, kind="ExternalInput")
    bnames = ["xlin_b1", "xlin_b2", "xcn_b1", "xcn_b2", "xij_b", "lin_b1"]
    ball_d = nc.dram_tensor("ball", [P, 2 * len(bnames)], f32,
                            kind="ExternalInput")
    lin_b2_d = nc.dram_tensor("lin_b2", [1, 1], f32, kind="ExternalInput")
    beta_d = nc.dram_tensor("beta_bc", [P, 1], f32, kind="ExternalInput")
    out_d = nc.dram_tensor("out", [1, EL], f32, kind="ExternalOutput")
    dbg = {}
    if DEBUG_DUMPS:
        dbg["h_all"] = nc.dram_tensor("dbg_h", [P, 2 * N], bf16,
                                      kind="ExternalOutput")
        dbg["cn"] = nc.dram_tensor("dbg_cn", [P, KC], bf16,
                                   kind="ExternalOutput")
        dbg["cnT"] = nc.dram_tensor("dbg_cnT", [P, KC], bf16,
                                    kind="ExternalOutput")
        dbg["xcn"] = nc.dram_tensor("dbg_xcn", [P, C], bf16,
                                    kind="ExternalOutput")
        dbg["xcnT"] = nc.dram_tensor("dbg_xcnT", [P, 2 * CGRP * P], bf16,
                                     kind="ExternalOutput")
        dbg["prodT"] = nc.dram_tensor("dbg_prodT", [P, 2 * CGRP * P], bf16,
                                      kind="ExternalOutput")

    _swq = [0]

    def _rr(inst):
        q = _swq[0] % 4
        _swq[0] += 1
        if q:
            inst.ins.queue = f"qPoolDynamic{q}"
        return inst

    with TileContext(nc) as tc:
        with (
            tc.tile_pool(name="const", bufs=1) as pK,
            tc.tile_pool(name="hpool", bufs=1) as pH,
            tc.tile_pool(name="adj", bufs=5) as pAdj,
            tc.tile_pool(name="cn", bufs=4) as pCn,
            tc.tile_pool(name="cnT", bufs=4) as pT,
            tc.tile_pool(name="edge", bufs=2) as pC,
            tc.tile_pool(name="xcn", bufs=CGRP) as pX,
        ):
            # ---- constants ----
            # idx first: the stage-B gathers depend only on these
            idx_sb = pK.tile([P, 2 * NB], i32, tag="idx_sb", name="idx_sb")
            nc.sync.dma_start(
                out=idx_sb[:].rearrange("p (t b) -> p t b", t=2),
                in_=idx_d[:, :].rearrange("t (b p) -> p t b", p=P),
            )
            ii = [idx_sb[:, b:b + 1] for b in range(NB)]
            jj = [idx_sb[:, NB + b:NB + b + 1] for b in range(NB)]

            wA_sb, wC_sb = {}, {}
            for n, t_d in list(wA.items()) + list(wC.items()):
                t = pK.tile([P, 2 * C], bf16, tag=f"w_{n}", name=f"w_{n}")
                nc.sync.dma_start(
                    out=t[:].rearrange("p (k n2) -> p k n2", k=2),
                    in_=t_d[:, :].rearrange("(k p) n2 -> p k n2", p=P),
                )
                pair = [t[:, 0:C], t[:, C:2 * C]]
                (wA_sb if n in wA else wC_sb)[n] = pair
            lw2_t = pK.tile([P, 2], bf16, tag="lin_w2", name="lin_w2t")
            nc.sync.dma_start(
                out=lw2_t[:].rearrange("p (k o) -> p k o", k=2),
                in_=lin_w2_d[:, :].rearrange("(k p) o -> p k o", p=P),
            )
            lw2_sb = [lw2_t[:, 0:1], lw2_t[:, 1:2]]
            b_sb = {}
            ball = pK.tile([P, 2 * len(bnames)], f32, tag="ball", name="ball")
            nc.sync.dma_start(
                out=ball[:],
                in_=ball_d[:, :],
            )
            for q, n in enumerate(bnames):
                b_sb[n] = ball[:, 2 * q:2 * q + 2]
            lb2_sb = pK.tile([1, 1], f32, tag="b_lin2", name="b_lin2")
            nc.sync.dma_start(out=lb2_sb[:], in_=lin_b2_d[:, :])
            beta_sb = pK.tile([P, 1], f32, tag="beta", name="beta")
            nc.sync.dma_start(out=beta_sb[:], in_=beta_d[:, :])

            out_row = pK.tile([1, EL], f32, tag="out_row", name="out_row")
            # natural-layout h in (hh, kt, c2) order so the xbar transposes
            # write contiguous per-partition spans: column = hh*N + kt*128 + c2
            # encodes h[node = kt*128 + p, channel = hh*128 + c2].
            h_all = pH.tile([P, 2 * N], bf16, tag="h_all", name="h_all")
            h_view = h_all[:].rearrange("p (hh kt c) -> p hh kt c", hh=2, c=P)

            # ---- stage definitions ----
            def stage_a_group(g, pA, psA):
                m0 = g * AGRP
                xT = []
                for h in range(2):
                    t = pA.tile([P, AGRP], bf16, tag=f"xT{h}", name=f"xT{h}_{g}")
                    nc.scalar.dma_start(
                        out=t[:], in_=xT_d[h * P:(h + 1) * P, m0:m0 + AGRP]
                    )
                    xT.append(t)
                y1T = []
                for h in range(2):
                    ps = psA.tile([P, AGRP], f32, tag="psmm", name=f"psA1_{g}{h}")
                    nc.tensor.matmul(
                        ps[:], wA_sb["xlin_w1"][0][:, h * P:(h + 1) * P],
                        xT[0][:], start=True, stop=False,
                    )
                    nc.tensor.matmul(
                        ps[:], wA_sb["xlin_w1"][1][:, h * P:(h + 1) * P],
                        xT[1][:], start=False, stop=True,
                    )
                    t = pA.tile([P, AGRP], bf16, tag=f"y1T{h}", name=f"y1T{h}_{g}")
                    nc.scalar.activation(
                        t[:], ps[:], Relu, bias=b_sb["xlin_b1"][:, h:h + 1]
                    )
                    y1T.append(t)
                for h in range(2):
                    ps = psA.tile([P, AGRP], f32, tag="psmm", name=f"psA2_{g}{h}")
                    nc.tensor.matmul(
                        ps[:], wA_sb["xlin_w2"][0][:, h * P:(h + 1) * P],
                        y1T[0][:], start=True, stop=False,
                    )
                    nc.tensor.matmul(
                        ps[:], wA_sb["xlin_w2"][1][:, h * P:(h + 1) * P],
                        y1T[1][:], start=False, stop=True,
                    )
                    y2 = pA.tile([P, AGRP], bf16, tag="y2T", name=f"y2T{h}_{g}")
                    nc.scalar.activation(
                        y2[:], ps[:], Relu, bias=b_sb["xlin_b2"][:, h:h + 1]
                    )
                    hT = pA.tile([P, AGRP], bf16, tag=f"hT{h}", name=f"hT{h}_{g}")
                    nc.vector.tensor_tensor(
                        out=hT[:], in0=xT[h][:], in1=y2[:], op=ADD
                    )
                    nc.sync.dma_start_transpose(
                        out=h_view[:, h,
                                   g * (AGRP // P):(g + 1) * (AGRP // P), :],
                        in_=hT[:],
                    )

            xcn_sb = [None] * NB

            cnT_map = {}

            def stage_b_load(b, s):
                ai = pAdj.tile([P, KC], fp8, tag="ai", name=f"ai{b}_{s}")
                _rr(nc.gpsimd.indirect_dma_start(
                    out=ai[:], out_offset=None, in_=adj_d[:, :],
                    in_offset=bass.IndirectOffsetOnAxis(ap=ii[b][:, :1], axis=0),
                    element_offset=s * KC,
                ))
                aj = pAdj.tile([P, KC], fp8, tag="aj", name=f"aj{b}_{s}")
                _rr(nc.gpsimd.indirect_dma_start(
                    out=aj[:], out_offset=None, in_=adj_d[:, :],
                    in_offset=bass.IndirectOffsetOnAxis(ap=jj[b][:, :1], axis=0),
                    element_offset=s * KC,
                ))
                cn = pCn.tile([P, KC], bf16, tag="cn", name=f"cn{b}_{s}")
                nc.vector.tensor_tensor(out=cn[:], in0=ai[:], in1=aj[:], op=MUL)
                cnT = pT.tile([P, KC], bf16, tag="cnT", name=f"cnT{b}_{s}")
                nc.sync.dma_start_transpose(
                    out=cnT[:].rearrange("p (kt e) -> p kt e", e=P),
                    in_=cn[:],
                )
                cnT_map[(b, s)] = cnT

            def stage_b_mms(b, s, psxcn):
                cnT = cnT_map[(b, s)]
                for kt in range(KC // P):
                    ktg = s * (KC // P) + kt
                    nc.tensor.matmul(
                        psxcn[:],
                        cnT[:, kt * P:(kt + 1) * P],
                        h_view[:, :, ktg, :],
                        start=(ktg == 0), stop=(ktg == NKT - 1),
                    )

            def stage_b_finish(b, psxcn):
                xcn_sb[b] = pX.tile([P, C], bf16, tag="xcn", name=f"xcn{b}")
                nc.vector.tensor_copy(xcn_sb[b][:], psxcn[:])

            prodT_map = {}

            def stage_c_prod(grp):
                blocks = range(grp * CGRP, (grp + 1) * CGRP)
                W = CGRP * P
                prodT = pC.tile([P, 2 * W], bf16, tag="prodT", name=f"prodT{grp}")
                prodT_v = prodT[:].rearrange(
                    "p (blk hh e) -> p blk hh e", blk=CGRP, e=P)
                prodT_map[grp] = prodT
                for t2, b in enumerate(blocks):
                    xi = pC.tile([P, C], bf16, tag="xi", name=f"xi{b}")
                    _rr(nc.gpsimd.indirect_dma_start(
                        out=xi[:], out_offset=None, in_=x_d[:, :],
                        in_offset=bass.IndirectOffsetOnAxis(
                            ap=ii[b][:, :1], axis=0),
                    ))
                    xj = pC.tile([P, C], bf16, tag="xj", name=f"xj{b}")
                    _rr(nc.gpsimd.indirect_dma_start(
                        out=xj[:], out_offset=None, in_=x_d[:, :],
                        in_offset=bass.IndirectOffsetOnAxis(
                            ap=jj[b][:, :1], axis=0),
                    ))
                    pt = pC.tile([P, C], bf16, tag="prod", name=f"prod{b}")
                    nc.vector.tensor_tensor(
                        out=pt[:], in0=xi[:], in1=xj[:], op=MUL
                    )
                    nc.sync.dma_start_transpose(
                        out=prodT_v[:, t2, :, :], in_=pt[:],
                    )

            def stage_c(grp, psC, psO):
                blocks = range(grp * CGRP, (grp + 1) * CGRP)
                W = CGRP * P  # 512 edges
                xcnT = pC.tile([P, 2 * W], bf16, tag="xcnT", name=f"xcnT{grp}")
                xcnT_v = xcnT[:].rearrange(
                    "p (blk hh e) -> p blk hh e", blk=CGRP, e=P)
                prodT = prodT_map[grp]
                for t2, b in enumerate(blocks):
                    nc.sync.dma_start_transpose(
                        out=xcnT_v[:, t2, :, :], in_=xcn_sb[b][:],
                    )

                def mlp_layer(rhs2, wname, bname, outtag, packed):
                    outs = []
                    for h in range(2):
                        ps = psC.tile([P, W], f32, tag="psc",
                                      name=f"psc_{grp}_{outtag}{h}")
                        if packed:
                            rhs_v = rhs2[:].rearrange(
                                "p (blk hh e) -> p blk hh e", blk=CGRP, e=P)
                            r0, r1 = rhs_v[:, :, 0, :], rhs_v[:, :, 1, :]
                        else:
                            r0, r1 = rhs2[0][:], rhs2[1][:]
                        nc.tensor.matmul(
                            ps[:], wC_sb[wname][0][:, h * P:(h + 1) * P],
                            r0, start=True, stop=False,
                        )
                        nc.tensor.matmul(
                            ps[:], wC_sb[wname][1][:, h * P:(h + 1) * P],
                            r1, start=False, stop=True,
                        )
                        t = pC.tile([P, W], bf16, tag=f"{outtag}{h}",
                                    name=f"{outtag}{h}_{grp}")
                        nc.scalar.activation(
                            t[:], ps[:], Relu, bias=b_sb[bname][:, h:h + 1]
                        )
                        outs.append(t)
                    return outs

                xijT = mlp_layer(prodT, "xij_w", "xij_b", "xijT", True)
                u1T = mlp_layer(xcnT, "xcn_w1", "xcn_b1", "u1T", True)
                u2T = mlp_layer(u1T, "xcn_w2", "xcn_b2", "u2T", False)
                zT = []
                for h in range(2):
                    zb = pC.tile([P, W], bf16, tag=f"zb{h}", name=f"zb{h}_{grp}")
                    nc.vector.tensor_tensor(
                        out=zb[:], in0=u2T[h][:],
                        in1=beta_sb[:, 0:1].to_broadcast([P, W]), op=MUL,
                    )
                    zt = pC.tile([P, W], bf16, tag=f"zT{h}", name=f"zT{h}_{grp}")
                    nc.vector.tensor_tensor(
                        out=zt[:], in0=zb[:], in1=xijT[h][:], op=ADD
                    )
                    zT.append(zt)
                vT = mlp_layer(zT, "lin_w1", "lin_b1", "vT", False)
                pso = psO.tile([1, W], f32, tag="pso", name=f"pso{grp}")
                nc.tensor.matmul(
                    pso[:], lw2_sb[0][:], vT[0][:], start=True, stop=False
                )
                nc.tensor.matmul(
                    pso[:], lw2_sb[1][:], vT[1][:], start=False, stop=True
                )
                nc.scalar.activation(
                    out_row[0:1, grp * W:(grp + 1) * W], pso[:],
                    Ident, bias=lb2_sb[0:1, 0:1],
                )

            # ---- software-pipelined emission ----
            with tc.tile_pool(name="psB", bufs=1, space="PSUM") as psB:
                ps_map = {}

                def open_half(bh):
                    for b in range(bh * CGRP, (bh + 1) * CGRP):
                        ps_map[b] = psB.tile(
                            [P, C], f32, tag=f"psxcn{b % CGRP}",
                            name=f"psxcn{b}")

                def b_loads(bh, s):
                    for b in range(bh * CGRP, (bh + 1) * CGRP):
                        stage_b_load(b, s)

                def b_mms(bh, s):
                    for b in range(bh * CGRP, (bh + 1) * CGRP):
                        stage_b_mms(b, s, ps_map[b])

                with tc.tile_pool(name="stA", bufs=3) as pA, \
                     tc.tile_pool(name="psA", bufs=4, space="PSUM") as psA:
                    open_half(0)
                    b_loads(0, 0)
                    for g in range(8):
                        stage_a_group(g, pA, psA)
                    b_mms(0, 0)
                    stage_c_prod(0)
                    b_loads(0, 1)
                    for g in range(8, 16):
                        stage_a_group(g, pA, psA)
                with tc.tile_pool(name="psC", bufs=2, space="PSUM") as psC, \
                     tc.tile_pool(name="psO", bufs=1, space="PSUM") as psO:
                    b_mms(0, 1)
                    stage_c_prod(1)
                    for b in range(CGRP):
                        stage_b_finish(b, ps_map[b])
                    open_half(1)
                    b_loads(1, 0)
                    b_mms(1, 0)
                    b_loads(1, 1)
                    stage_c(0, psC, psO)
                    b_mms(1, 1)
                    for b in range(CGRP, 2 * CGRP):
                        stage_b_finish(b, ps_map[b])
                    stage_c(1, psC, psO)

            nc.sync.dma_start(out=out_d[:, :], in_=out_row[0:1, :])
            if DEBUG_DUMPS:
                nc.sync.dma_start(out=dbg["h_all"][:, :], in_=h_all[:])

    return _split_multi_waits(nc) if split_waits else nc


def kernel(**inputs):
    from concourse.bass_utils import run_bass_kernel_spmd

    if "nc" not in _CACHE:
        _CACHE["nc"] = _build()
    nc = _CACHE["nc"]

    x = np.ascontiguousarray(inputs["x"], dtype=np.float32)
    adj8 = np.ascontiguousarray(inputs["adj"]).astype(ml_dtypes.float8_e4m3)
    tar = np.asarray(inputs["tar_ei"]).astype(np.int32)

    def btile(b):
        return np.ascontiguousarray(np.asarray(b, dtype=np.float32).reshape(2, P).T)

    common = {
        "x": x.astype(ml_dtypes.bfloat16),
        "xT": np.ascontiguousarray(x.T).astype(ml_dtypes.bfloat16),
        "adj": adj8,
        "beta_bc": np.full((P, 1), np.asarray(inputs["beta"]).reshape(-1)[0],
                           dtype=np.float32),
        "lin_w2": np.ascontiguousarray(inputs["lin_w2"]).astype(ml_dtypes.bfloat16),
        "lin_b2": np.asarray(inputs["lin_b2"], dtype=np.float32).reshape(1, 1),
    }
    for n in ("xlin_w1", "xlin_w2"):
        common[n] = np.ascontiguousarray(inputs[n]).astype(ml_dtypes.bfloat16)
    for n in ("xcn_w1", "xcn_w2", "xij_w", "lin_w1"):
        common[n] = np.ascontiguousarray(inputs[n]).astype(ml_dtypes.bfloat16)
    common["ball"] = np.ascontiguousarray(np.concatenate(
        [btile(inputs[n]) for n in
         ("xlin_b1", "xlin_b2", "xcn_b1", "xcn_b2", "xij_b", "lin_b1")],
        axis=1))

    in_maps = []
    for c in range(NCORES):
        m = dict(common)
        m["idx"] = np.ascontiguousarray(tar[:, c * EL:(c + 1) * EL])
        in_maps.append(m)

    res = run_bass_kernel_spmd(
        nc, in_maps, core_ids=list(range(NCORES)), trace=TRACE
    )
    global LAST_RESULT
    LAST_RESULT = res
    out = np.concatenate(
        [res.results[c]["out"].reshape(EL, 1) for c in range(NCORES)], axis=0
    )
    return out.astype(np.float32)



# revision 15
# speedup vs baseline: 1.4281x; 1.4281x over previous
"""CNLinkPredictor Trainium2 kernel (fp8 DoubleRow common-neighbor pipeline).

Edge-sharded across 8 NeuronCores (1024 target edges each); x, adj, and the
MLP weights are replicated. Per core:

  A) h = x + MLP(x), finishing in NATURAL (node-partition) layout so the
     result lands directly in the fp8 block-major layout the DoubleRow
     matmul wants for its stationary operand:
       - L1 stays transposed: y1T = relu(W1^T xT) (fp8 weights + moving).
       - L2 flips orientation per 128-node tile: psum[node, c] accumulates
         y1T-chunks as stationary with W2 moving; the bias lands via a K=1
         ones-row x b2-row matmul; relu on ACT; DVE adds the x residual and
         writes fp8 straight into h8[p, T*256:(T+1)*256].
  B) per 128-edge block: one indirect full-row gather per endpoint from a
     column-shuffled adjacency (host prep), uint16 bitwise-AND (exact for
     0/1 fp8 patterns, runs at the 2-byte DVE rate), two uint16 packed
     transposes (half the xbar tile count of bf16), then 64 DoubleRow fp8
     matmuls accumulating xcnT[c, e] directly - the adjacency column
     shuffle makes the transpose pairing k=2p+j line up with h8's block
     slots.
  C) edge MLPs in transposed layout (bf16): xcnT comes straight out of B's
     PSUM (no transpose), xi*xj is gathered/multiplied/transposed as before.

Hardware pitfalls this kernel works around (carried from the previous
session, all re-validated):
  - walrus accepts at most ONE sync-wait per instruction
    (_apply_tile_patch + _split_multi_waits).
  - Concurrent 4-byte DMA traffic corrupts in-flight 2-byte xbar
    DMA-transposes: every steady-state transfer is <= 2 bytes/element;
    f32/i32 loads happen up front, the single f32 store happens last.
  - xbar transposes need contiguous per-partition destinations.
  - DoubleRow needs a block-major stationary operand (pair step % 16 == 0);
    the byte-interleaved transpose output is only legal as the MOVING
    operand (verified empirically - the ISA check rejects it as weights).
  - PSUM zero regions are 2048 B: accumulation groups sharing a psum tile
    run strictly block-major so a start=True never clobbers a neighbor.
"""

import numpy as np
import ml_dtypes

N = 8192
C = 256
E = 8192
NCORES = 8
EL = E // NCORES          # edges per core
P = 128
NB = EL // P              # edge blocks per core (8)
NCHUNK = N // 256         # 256-node DoubleRow chunks (32)
NT = N // P               # stage-A node tiles (64)
AGRP = 512                # stage-A node group (4 tiles)
NG = N // AGRP            # stage-A groups (16)
CGRP = 4                  # stage-C blocks per group (512 edges)

_CACHE = {}
TRACE = False
LAST_RESULT = None


def _apply_tile_patch():
    """Split the Tile tail-drain's multi-sem wait onto individual SP nops."""
    from concourse.tile import TileContext
    from concourse.vector_clock import ScopedClock

    if getattr(TileContext, "_drain_patched", False):
        return

    def _patched(self, tick_clock, wait_clock):
        nc = self.nc
        collector = nc.sync.nop()
        wait_clock.add_sem_waits(
            collector.ins, ScopedClock({None: tick_clock.global_clock})
        )
        si = collector.ins.sync_info
        waits = list(si.on_wait) if si is not None and si.on_wait else []
        if si is not None and len(waits) > 1:
            name_to_handle = {h.name: h for h in self.sems.allocated().values()}
            si.on_wait = [waits[0]]
            for w in waits[1:]:
                op = {
                    "sem-ge-imm": "sem-ge",
                    "sem-eq-imm": "sem-eq",
                    "sem-le-imm": "sem-le",
                }.get(str(w.wait_mode), "sem-ge")
                nc.sync.nop().wait_op(name_to_handle[w.ant_name], w.wait_value, op)
        nc.sync.drain()
        nc.all_engine_barrier()
        assert self.sems is not None
        popped = nc._tile_sem_poison_stack.pop()
        assert popped is self._sem_poison
        nc.clear_and_free_semaphores(list(self.sems.allocated().values()))
        nc.all_engine_barrier()

    TileContext._drain_and_barrier = _patched
    TileContext._drain_patched = True


def _split_multi_waits(nc):
    """Hoist extra sync-waits onto same-engine NoOps (sequential waits ==
    ANDed waits); this walrus build allows one wait per instruction."""
    import concourse.mybir as mybir

    cnt = 0
    for fn in nc.m.functions:
        for bb in fn.blocks:
            out = []
            for inst in bb.instructions:
                si = getattr(inst, "sync_info", None)
                waits = list(si.on_wait) if si is not None and si.on_wait else []
                if len(waits) > 1:
                    for w in waits[:-1]:
                        nop = mybir.InstNoOp(name=f"ws-{cnt}", ins=[], outs=[])
                        cnt += 1
                        nop.engine = inst.engine
                        nop.sync_info = mybir.SyncInfo(on_wait=[w], on_update=[])
                        out.append(nop)
                    si.on_wait = [waits[-1]]
                out.append(inst)
            bb.instructions = out
    return nc


def _build(split_waits=True):
    import concourse.bass as bass
    import concourse.mybir as mybir
    from concourse.tile import TileContext

    _apply_tile_patch()

    f32 = mybir.dt.float32
    bf16 = mybir.dt.bfloat16
    fp8 = mybir.dt.float8e4
    u16 = mybir.dt.uint16
    i32 = mybir.dt.int32
    Relu = mybir.ActivationFunctionType.Relu
    Ident = mybir.ActivationFunctionType.Identity
    MUL = mybir.AluOpType.mult
    ADD = mybir.AluOpType.add
    AND = mybir.AluOpType.bitwise_and
    DR = mybir.MatmulPerfMode.DoubleRow

    nc = bass.Bass(num_swdge_queues=4, dynamic_dma_scratch_size=32768)

    # host-pretiled: xa8[p, k*N + n] = x[n, k*128 + p] (fp8)
    xa8_d = nc.dram_tensor("xa8", [P, 2 * N], fp8, kind="ExternalInput")
    # host-pretiled: xr8[p, T*C + c] = x[T*128 + p, c] (fp8)
    xr8_d = nc.dram_tensor("xr8t", [P, 2 * N], fp8, kind="ExternalInput")
    x_d = nc.dram_tensor("x", [N, C], bf16, kind="ExternalInput")
    adjs_d = nc.dram_tensor("adjs", [N, N], fp8, kind="ExternalInput")
    idx_d = nc.dram_tensor("idx", [2, EL], i32, kind="ExternalInput")
    # host-packed fp8 stage-A weights: [p, (which 2, k 2, cout 256)] + ones/b2
    wa8_d = nc.dram_tensor("wa8", [P, 4 * C], fp8, kind="ExternalInput")
    onesb2_d = nc.dram_tensor("onesb2", [1, P + C], fp8, kind="ExternalInput")
    # host-packed bf16 stage-C weights: [p, (which 4, k 2, cout 256)]
    wc_d = nc.dram_tensor("wc", [P, 8 * C], bf16, kind="ExternalInput")
    lin_w2_d = nc.dram_tensor("lin_w2", [C, 1], bf16, kind="ExternalInput")
    bnames = ["xlin_b1", "xcn_b1", "xcn_b2", "xij_b", "lin_b1"]
    # host-packed f32: [p, (bias pairs 10, beta 1, lin_b2 1)]
    fpk_d = nc.dram_tensor("fpk", [P, 2 * len(bnames) + 2], f32,
                           kind="ExternalInput")
    out_d = nc.dram_tensor("out", [1, EL], f32, kind="ExternalOutput")

    _swq = [0]

    def _rr(inst):
        q = _swq[0] % 4
        _swq[0] += 1
        if q:
            inst.ins.queue = f"qPoolDynamic{q}"
        return inst

    with TileContext(nc) as tc:
        with (
            tc.tile_pool(name="const", bufs=1) as pK,
            tc.tile_pool(name="h8p", bufs=1) as pH,
            tc.tile_pool(name="adj", bufs=2) as pAdj,
            tc.tile_pool(name="cn", bufs=1) as pCn,
            tc.tile_pool(name="cnT", bufs=NB - 1) as pT,
            tc.tile_pool(name="xcnT", bufs=1) as pXT,
            tc.tile_pool(name="prod", bufs=2) as pPr,
            tc.tile_pool(name="edge", bufs=1) as pC,
        ):
            # ---- constants (f32/i32 first: they must finish before the
            # first 2-byte xbar transpose is in flight) ----
            idx_sb = pK.tile([P, 2 * NB], i32, tag="idx_sb", name="idx_sb")
            nc.sync.dma_start(
                out=idx_sb[:].rearrange("p (t b) -> p t b", t=2),
                in_=idx_d[:, :].rearrange("t (b p) -> p t b", p=P),
            )
            ii = [idx_sb[:, b:b + 1] for b in range(NB)]
            jj = [idx_sb[:, NB + b:NB + b + 1] for b in range(NB)]

            fpk = pK.tile([P, 2 * len(bnames) + 2], f32, tag="fpk",
                          name="fpk")
            nc.sync.dma_start(out=fpk[:], in_=fpk_d[:, :])
            b_sb = {}
            for q, n in enumerate(bnames):
                b_sb[n] = fpk[:, 2 * q:2 * q + 2]
            beta_sb = fpk[:, 10:11]
            lb2_sb = fpk[:, 11:12]

            wa8 = pK.tile([P, 4 * C], fp8, tag="wa8", name="wa8")
            nc.sync.dma_start(out=wa8[:], in_=wa8_d[:, :])
            w1_sb = wa8[:, 0:2 * C]
            w2_sb = wa8[:, 2 * C:4 * C]
            onesb2 = pK.tile([1, P + C], fp8, tag="onesb2", name="onesb2")
            nc.sync.dma_start(out=onesb2[:], in_=onesb2_d[:, :])
            ones_sb = onesb2[:, 0:P]
            b2row_sb = onesb2[:, P:P + C]

            wc_t = pK.tile([P, 8 * C], bf16, tag="wc", name="wc")
            nc.sync.dma_start(out=wc_t[:], in_=wc_d[:, :])
            wC_sb = {}
            for q, n in enumerate(("xcn_w1", "xcn_w2", "xij_w", "lin_w1")):
                wC_sb[n] = [wc_t[:, q * 2 * C:q * 2 * C + C],
                            wc_t[:, q * 2 * C + C:(q + 1) * 2 * C]]
            lw2_t = pK.tile([P, 2], bf16, tag="lin_w2", name="lin_w2t")
            nc.sync.dma_start(
                out=lw2_t[:].rearrange("p (k o) -> p k o", k=2),
                in_=lin_w2_d[:, :].rearrange("(k p) o -> p k o", p=P),
            )
            lw2_sb = [lw2_t[:, 0:1], lw2_t[:, 1:2]]

            # the two big stage-A input slabs: one DMA each (host-pretiled)
            xa8 = pK.tile([P, 2 * N], fp8, tag="xa8", name="xa8")
            nc.sync.dma_start(out=xa8[:], in_=xa8_d[:, :])
            xr8t = pK.tile([P, 2 * N], fp8, tag="xr8t", name="xr8t")
            nc.sync.dma_start(out=xr8t[:], in_=xr8_d[:, :])

            out_row = pK.tile([1, EL], f32, tag="out_row", name="out_row")

            # h8[p, T*256 + c] = h[node 128*T + p, channel c] in fp8.
            # DoubleRow stationary slice (chunk, ch): [p][j: stride 256]
            # [c2: 128 contiguous] at offset chunk*512 + ch*128.
            h8 = pH.tile([P, 2 * N], fp8, tag="h8", name="h8")
            h8_v = h8[:].rearrange(
                "p (ck j ch c2) -> p ck ch j c2", ck=NCHUNK, j=2, ch=2)

            # ---- stage B state ----
            cnT = [None] * NB
            xcnT_sb = [
                pXT.tile([P, EL], bf16, tag=f"xcnT{ch}", name=f"xcnT{ch}")
                for ch in range(2)
            ]

            def b_gather(b, which):
                t = pAdj.tile([P, N], fp8, tag=f"a{which}", name=f"a{which}{b}")
                off = (ii if which == "i" else jj)[b]
                _rr(nc.gpsimd.indirect_dma_start(
                    out=t[:], out_offset=None, in_=adjs_d[:, :],
                    in_offset=bass.IndirectOffsetOnAxis(ap=off[:, :1], axis=0),
                ))
                return t

            def b_and_transpose(b, ai, aj):
                cn8 = pCn.tile([P, N // 2], u16, tag="cn8", name=f"cn8_{b}")
                nc.vector.tensor_tensor(
                    out=cn8[:], in0=ai[:].bitcast(u16),
                    in1=aj[:].bitcast(u16), op=AND,
                )
                cnT[b] = pT.tile([P, N // 2], u16, tag="cnT", name=f"cnT{b}")
                nc.sync.dma_start_transpose(
                    out=cnT[b][:].rearrange("p (cl e) -> p cl e", e=P),
                    in_=cn8[:],
                )

            def b_matmuls(b, psT):
                rhs_v = cnT[b][:].bitcast(fp8).rearrange(
                    "p (ck e j) -> p ck j e", ck=NCHUNK, j=2)
                for ch in range(2):
                    o = psT[ch][:, b * P:(b + 1) * P]
                    for chunk in range(NCHUNK):
                        nc.tensor.matmul(
                            o, h8_v[:, chunk, ch, :, :], rhs_v[:, chunk, :, :],
                            start=(chunk == 0), stop=(chunk == NCHUNK - 1),
                            perf_mode=DR,
                        )

            def b_copyout(b, psT):
                for ch in range(2):
                    nc.vector.tensor_copy(
                        xcnT_sb[ch][:, b * P:(b + 1) * P],
                        psT[ch][:, b * P:(b + 1) * P],
                    )

            # ---- stage C (transposed-layout edge MLPs, bf16) ----
            prodT_map = {}

            def stage_c_prod(grp):
                W = CGRP * P
                prodT = pPr.tile([P, 2 * W], bf16, tag="prodT", name=f"prodT{grp}")
                prodT_v = prodT[:].rearrange(
                    "p (blk hh e) -> p blk hh e", blk=CGRP, e=P)
                prodT_map[grp] = prodT
                for t2, b in enumerate(range(grp * CGRP, (grp + 1) * CGRP)):
                    xi = pPr.tile([P, C], bf16, tag="xi", name=f"xi{b}")
                    _rr(nc.gpsimd.indirect_dma_start(
                        out=xi[:], out_offset=None, in_=x_d[:, :],
                        in_offset=bass.IndirectOffsetOnAxis(
                            ap=ii[b][:, :1], axis=0),
                    ))
                    xj = pPr.tile([P, C], bf16, tag="xj", name=f"xj{b}")
                    _rr(nc.gpsimd.indirect_dma_start(
                        out=xj[:], out_offset=None, in_=x_d[:, :],
                        in_offset=bass.IndirectOffsetOnAxis(
                            ap=jj[b][:, :1], axis=0),
                    ))
                    pt = pPr.tile([P, C], bf16, tag="prod", name=f"prod{b}")
                    nc.vector.tensor_tensor(
                        out=pt[:], in0=xi[:], in1=xj[:], op=MUL
                    )
                    nc.sync.dma_start_transpose(
                        out=prodT_v[:, t2, :, :], in_=pt[:],
                    )

            def stage_c(grp, psC, psO):
                W = CGRP * P  # 512 edges

                def mlp_layer(pair, wname, bname, outtag, packed=False):
                    outs = []
                    for h in range(2):
                        ps = psC.tile([P, W], f32, tag="psc",
                                      name=f"psc_{grp}_{outtag}{h}")
                        if packed:
                            rhs_v = pair[:].rearrange(
                                "p (blk hh e) -> p blk hh e", blk=CGRP, e=P)
                            r0, r1 = rhs_v[:, :, 0, :], rhs_v[:, :, 1, :]
                        else:
                            r0, r1 = pair
                        nc.tensor.matmul(
                            ps[:], wC_sb[wname][0][:, h * P:(h + 1) * P],
                            r0, start=True, stop=False,
                        )
                        nc.tensor.matmul(
                            ps[:], wC_sb[wname][1][:, h * P:(h + 1) * P],
                            r1, start=False, stop=True,
                        )
                        t = pC.tile([P, W], bf16, tag=f"{outtag}{h}",
                                    name=f"{outtag}{h}_{grp}")
                        nc.scalar.activation(
                            t[:], ps[:], Relu, bias=b_sb[bname][:, h:h + 1]
                        )
                        outs.append(t)
                    return outs

                xijT = mlp_layer(prodT_map[grp], "xij_w", "xij_b", "xijT",
                                 packed=True)
                xcn_pair = (xcnT_sb[0][:, grp * W:(grp + 1) * W],
                            xcnT_sb[1][:, grp * W:(grp + 1) * W])
                u1T = mlp_layer(xcn_pair, "xcn_w1", "xcn_b1", "u1T")
                u2T = mlp_layer([u1T[0][:], u1T[1][:]], "xcn_w2", "xcn_b2",
                                "u2T")
                zT = []
                for h in range(2):
                    zb = pC.tile([P, W], bf16, tag=f"zb{h}", name=f"zb{h}_{grp}")
                    nc.vector.tensor_tensor(
                        out=zb[:], in0=u2T[h][:],
                        in1=beta_sb[:, 0:1].to_broadcast([P, W]), op=MUL,
                    )
                    zt = pC.tile([P, W], bf16, tag=f"zT{h}", name=f"zT{h}_{grp}")
                    nc.vector.tensor_tensor(
                        out=zt[:], in0=zb[:], in1=xijT[h][:], op=ADD
                    )
                    zT.append(zt)
                vT = mlp_layer([zT[0][:], zT[1][:]], "lin_w1", "lin_b1", "vT")
                pso = psO.tile([1, W], f32, tag="pso", name=f"pso{grp}")
                nc.tensor.matmul(
                    pso[:], lw2_sb[0][:], vT[0][:], start=True, stop=False
                )
                nc.tensor.matmul(
                    pso[:], lw2_sb[1][:], vT[1][:], start=False, stop=True
                )
                nc.scalar.activation(
                    out_row[0:1, grp * W:(grp + 1) * W], pso[:],
                    Ident, bias=lb2_sb[0:1, 0:1],
                )

            # ---- emission ----
            with tc.tile_pool(name="psB", bufs=1, space="PSUM") as psBp:
                psT = [
                    psBp.tile([P, EL], f32, tag=f"psT{ch}", name=f"psT{ch}")
                    for ch in range(2)
                ]

                gathered = {}
                with tc.tile_pool(name="stA", bufs=4) as pA, \
                     tc.tile_pool(name="psA", bufs=2, space="PSUM") as psA, \
                     tc.tile_pool(name="psL2", bufs=2, space="PSUM") as psL2:
                    for g in range(NG):
                        b = g // 2
                        if g % 2 == 0:
                            gathered[b] = [b_gather(b, "i")]
                        else:
                            gathered[b].append(b_gather(b, "j"))

                        m0 = g * AGRP
                        y1T = []
                        for ch in range(2):
                            ps = psA.tile([P, AGRP], f32, tag="psA",
                                          name=f"psA_{g}{ch}")
                            nc.tensor.matmul(
                                ps[:], w1_sb[:, ch * P:(ch + 1) * P],
                                xa8[:, m0:m0 + AGRP],
                                start=True, stop=False,
                            )
                            nc.tensor.matmul(
                                ps[:], w1_sb[:, C + ch * P:C + (ch + 1) * P],
                                xa8[:, N + m0:N + m0 + AGRP],
                                start=False, stop=True,
                            )
                            t = pA.tile([P, AGRP], fp8, tag=f"y1T{ch}",
                                        name=f"y1T{ch}_{g}")
                            nc.scalar.activation(
                                t[:], ps[:], Relu,
                                bias=b_sb["xlin_b1"][:, ch:ch + 1],
                            )
                            y1T.append(t)
                        for t2 in range(4):
                            T = 4 * g + t2
                            ps2 = psL2.tile([P, C], f32, tag="psL2",
                                            name=f"psL2_{T}")
                            nc.tensor.matmul(
                                ps2[:], y1T[0][:, t2 * P:(t2 + 1) * P],
                                w2_sb[:, 0:C], start=True, stop=False,
                            )
                            nc.tensor.matmul(
                                ps2[:], y1T[1][:, t2 * P:(t2 + 1) * P],
                                w2_sb[:, C:2 * C], start=False, stop=False,
                            )
                            nc.tensor.matmul(
                                ps2[:], ones_sb[0:1, :], b2row_sb[0:1, :],
                                start=False, stop=True,
                            )
                            rel = pA.tile([P, C], bf16, tag="rel",
                                          name=f"rel_{T}")
                            nc.scalar.activation(rel[:], ps2[:], Relu)
                            nc.vector.tensor_tensor(
                                out=h8[:, T * C:(T + 1) * C],
                                in0=xr8t[:, T * C:(T + 1) * C],
                                in1=rel[:], op=ADD,
                            )
                        if g % 2 == 1 and len(gathered[b]) == 2:
                            b_and_transpose(b, *gathered.pop(b))

                with tc.tile_pool(name="psC", bufs=2, space="PSUM") as psC, \
                     tc.tile_pool(name="psO", bufs=1, space="PSUM") as psO:
                    for b in range(NB):
                        if b == 0:
                            stage_c_prod(0)
                        if b == 2:
                            stage_c_prod(1)
                        b_matmuls(b, psT)
                        b_copyout(b, psT)
                        if b == CGRP - 1:
                            stage_c(0, psC, psO)
                        if b == NB - 1:
                            stage_c(1, psC, psO)

            nc.sync.dma_start(out=out_d[:, :], in_=out_row[0:1, :])

    return _split_multi_waits(nc) if split_waits else nc


def _col_shuffle_perm():
    """d[m]: DRAM column position for original node m so the cnT transpose
    pairing (k = 2p + j) matches h8's block-major slots (node 128T + p at
    chunk T//2, j = T%2)."""
    m = np.arange(N)
    T = m // P
    p = m % P
    return 256 * (T // 2) + 2 * p + (T % 2)


def kernel(**inputs):
    from concourse.bass_utils import run_bass_kernel_spmd

    if "nc" not in _CACHE:
        _CACHE["nc"] = _build()
    nc = _CACHE["nc"]

    x = np.ascontiguousarray(inputs["x"], dtype=np.float32)
    adj8 = np.ascontiguousarray(inputs["adj"]).astype(ml_dtypes.float8_e4m3)
    d = _col_shuffle_perm()
    adjs = np.empty_like(adj8)
    adjs[:, d] = adj8
    tar = np.asarray(inputs["tar_ei"]).astype(np.int32)

    x8 = x.astype(ml_dtypes.float8_e4m3)
    # xa8[p, k*N + n] = x[n, k*128 + p]
    xa8 = np.ascontiguousarray(
        x8.T.reshape(2, P, N).transpose(1, 0, 2).reshape(P, 2 * N))
    # xr8t[p, T*C + c] = x[T*128 + p, c]
    xr8t = np.ascontiguousarray(
        x8.reshape(NT, P, C).transpose(1, 0, 2).reshape(P, NT * C))

    def wtile(w, dt):
        # [p, (ksub 2, cout C)] from [C, C]
        return np.ascontiguousarray(
            np.asarray(w).astype(dt).reshape(2, P, C).transpose(1, 0, 2)
            .reshape(P, 2 * C))

    wa8 = np.concatenate(
        [wtile(inputs["xlin_w1"], ml_dtypes.float8_e4m3),
         wtile(inputs["xlin_w2"], ml_dtypes.float8_e4m3)], axis=1)
    onesb2 = np.concatenate(
        [np.ones((1, P), np.float32),
         np.asarray(inputs["xlin_b2"], np.float32).reshape(1, C)],
        axis=1).astype(ml_dtypes.float8_e4m3)
    wc = np.concatenate(
        [wtile(inputs[n], ml_dtypes.bfloat16)
         for n in ("xcn_w1", "xcn_w2", "xij_w", "lin_w1")], axis=1)

    def btile(b):
        return np.ascontiguousarray(
            np.asarray(b, dtype=np.float32).reshape(2, P).T)

    fpk = np.concatenate(
        [btile(inputs[n]) for n in
         ("xlin_b1", "xcn_b1", "xcn_b2", "xij_b", "lin_b1")] +
        [np.full((P, 1), np.asarray(inputs["beta"]).reshape(-1)[0],
                 dtype=np.float32),
         np.full((P, 1), np.asarray(inputs["lin_b2"]).reshape(-1)[0],
                 dtype=np.float32)],
        axis=1)

    common = {
        "x": x.astype(ml_dtypes.bfloat16),
        "xa8": xa8,
        "xr8t": xr8t,
        "adjs": adjs,
        "wa8": wa8,
        "onesb2": onesb2,
        "wc": wc,
        "fpk": fpk,
        "lin_w2": np.ascontiguousarray(inputs["lin_w2"]).astype(
            ml_dtypes.bfloat16),
    }

    in_maps = []
    for c in range(NCORES):
        m = dict(common)
        m["idx"] = np.ascontiguousarray(tar[:, c * EL:(c + 1) * EL])
        in_maps.append(m)

    res = run_bass_kernel_spmd(
        nc, in_maps, core_ids=list(range(NCORES)), trace=TRACE
    )
    global LAST_RESULT
    LAST_RESULT = res
    out = np.concatenate(
        [res.results[c]["out"].reshape(EL, 1) for c in range(NCORES)], axis=0
    )
    return out.astype(np.float32)


# revision 39
# speedup vs baseline: 1.7984x; 1.2593x over previous
"""CNLinkPredictor Trainium2 kernel (fp8 DoubleRow common-neighbor pipeline).

Edge-sharded across 8 NeuronCores (1024 target edges each); x, adj, and the
MLP weights are replicated. Per core:

  A) h = x + MLP(x), finishing in NATURAL (node-partition) layout so the
     result lands directly in the fp8 block-major layout the DoubleRow
     matmul wants for its stationary operand:
       - L1 stays transposed: y1T = relu(W1^T xT) (fp8 weights + moving).
       - L2 flips orientation per 128-node tile: psum[node, c] accumulates
         y1T-chunks as stationary with W2 moving; the bias lands via a K=1
         ones-row x b2-row matmul; relu on ACT; DVE adds the x residual and
         writes fp8 straight into h8[p, T*256:(T+1)*256].
  B) per 128-edge block: one indirect full-row gather per endpoint from a
     column-shuffled adjacency (host prep), uint16 bitwise-AND (exact for
     0/1 fp8 patterns, runs at the 2-byte DVE rate), two uint16 packed
     transposes (half the xbar tile count of bf16), then 64 DoubleRow fp8
     matmuls accumulating xcnT[c, e] directly - the adjacency column
     shuffle makes the transpose pairing k=2p+j line up with h8's block
     slots.
  C) edge MLPs in transposed layout (bf16): xcnT comes straight out of B's
     PSUM (no transpose), xi*xj is gathered/multiplied/transposed as before.

Hardware pitfalls this kernel works around (carried from the previous
session, all re-validated):
  - walrus accepts at most ONE sync-wait per instruction
    (_apply_tile_patch + _split_multi_waits).
  - Concurrent 4-byte DMA traffic corrupts in-flight 2-byte xbar
    DMA-transposes: every steady-state transfer is <= 2 bytes/element;
    f32/i32 loads happen up front, the single f32 store happens last.
  - xbar transposes need contiguous per-partition destinations.
  - DoubleRow needs a block-major stationary operand (pair step % 16 == 0);
    the byte-interleaved transpose output is only legal as the MOVING
    operand (verified empirically - the ISA check rejects it as weights).
  - PSUM zero regions are 2048 B: accumulation groups sharing a psum tile
    run strictly block-major so a start=True never clobbers a neighbor.
"""

import numpy as np
import ml_dtypes

N = 8192
C = 256
E = 8192
NCORES = 8
EL = E // NCORES          # edges per core
P = 128
NB = EL // P              # edge blocks per core (8)
NCHUNK = N // 256         # 256-node DoubleRow chunks (32)
NT = N // P               # stage-A node tiles (64)
AGRP = 512                # stage-A node group (4 tiles)
NG = N // AGRP            # stage-A groups (16)
CGRP = 4                  # stage-C blocks per group (512 edges)

_CACHE = {}
TRACE = False
LAST_RESULT = None


def _apply_tile_patch():
    """Split the Tile tail-drain's multi-sem wait onto individual SP nops."""
    from concourse.tile import TileContext
    from concourse.vector_clock import ScopedClock

    if getattr(TileContext, "_drain_patched", False):
        return

    def _patched(self, tick_clock, wait_clock):
        nc = self.nc
        collector = nc.sync.nop()
        wait_clock.add_sem_waits(
            collector.ins, ScopedClock({None: tick_clock.global_clock})
        )
        si = collector.ins.sync_info
        waits = list(si.on_wait) if si is not None and si.on_wait else []
        if si is not None and len(waits) > 1:
            name_to_handle = {h.name: h for h in self.sems.allocated().values()}
            si.on_wait = [waits[0]]
            for w in waits[1:]:
                op = {
                    "sem-ge-imm": "sem-ge",
                    "sem-eq-imm": "sem-eq",
                    "sem-le-imm": "sem-le",
                }.get(str(w.wait_mode), "sem-ge")
                nc.sync.nop().wait_op(name_to_handle[w.ant_name], w.wait_value, op)
        nc.sync.drain()
        nc.all_engine_barrier()
        assert self.sems is not None
        popped = nc._tile_sem_poison_stack.pop()
        assert popped is self._sem_poison
        nc.clear_and_free_semaphores(list(self.sems.allocated().values()))
        nc.all_engine_barrier()

    TileContext._drain_and_barrier = _patched
    TileContext._drain_patched = True


def _split_multi_waits(nc):
    """Hoist extra sync-waits onto same-engine NoOps (sequential waits ==
    ANDed waits); this walrus build allows one wait per instruction."""
    import concourse.mybir as mybir

    cnt = 0
    for fn in nc.m.functions:
        for bb in fn.blocks:
            out = []
            for inst in bb.instructions:
                si = getattr(inst, "sync_info", None)
                waits = list(si.on_wait) if si is not None and si.on_wait else []
                if len(waits) > 1:
                    for w in waits[:-1]:
                        nop = mybir.InstNoOp(name=f"ws-{cnt}", ins=[], outs=[])
                        cnt += 1
                        nop.engine = inst.engine
                        nop.sync_info = mybir.SyncInfo(on_wait=[w], on_update=[])
                        out.append(nop)
                    si.on_wait = [waits[-1]]
                out.append(inst)
            bb.instructions = out
    return nc


def _build(split_waits=True):
    import concourse.bass as bass
    import concourse.mybir as mybir
    from concourse.tile import TileContext

    _apply_tile_patch()

    f32 = mybir.dt.float32
    bf16 = mybir.dt.bfloat16
    fp8 = mybir.dt.float8e4
    u16 = mybir.dt.uint16
    i32 = mybir.dt.int32
    Relu = mybir.ActivationFunctionType.Relu
    Ident = mybir.ActivationFunctionType.Identity
    MUL = mybir.AluOpType.mult
    ADD = mybir.AluOpType.add
    AND = mybir.AluOpType.bitwise_and
    DR = mybir.MatmulPerfMode.DoubleRow

    nc = bass.Bass(num_swdge_queues=4, dynamic_dma_scratch_size=32768)

    # host-pretiled: xa8[p, k*N + n] = x[n, k*128 + p] (fp8)
    xa8_d = nc.dram_tensor("xa8", [P, 2 * N], fp8, kind="ExternalInput")
    # host-pretiled: xr8[p, T*C + c] = x[T*128 + p, c] (fp8)
    xr8_d = nc.dram_tensor("xr8t", [P, 2 * N], fp8, kind="ExternalInput")
    x_d = nc.dram_tensor("x", [N, C], bf16, kind="ExternalInput")
    adjs_d = nc.dram_tensor("adjs", [N, N], fp8, kind="ExternalInput")
    idx_d = nc.dram_tensor("idx", [2, EL], i32, kind="ExternalInput")
    # host-packed fp8 stage-A weights: [p, (which 2, k 2, cout 256)] + ones/b2
    wa8_d = nc.dram_tensor("wa8", [P, 4 * C], fp8, kind="ExternalInput")
    onesb2_d = nc.dram_tensor("onesb2", [1, P + C], fp8, kind="ExternalInput")
    # host-packed bf16 stage-C weights: [p, (which 4, k 2, cout 256)]
    wc_d = nc.dram_tensor("wc", [P, 8 * C], bf16, kind="ExternalInput")
    lin_w2_d = nc.dram_tensor("lin_w2", [C, 1], bf16, kind="ExternalInput")
    bnames = ["xlin_b1", "xcn_b1", "xcn_b2", "xij_b", "lin_b1"]
    # host-packed f32: [p, (bias pairs 10, beta 1, lin_b2 1)]
    fpk_d = nc.dram_tensor("fpk", [P, 2 * len(bnames) + 2], f32,
                           kind="ExternalInput")
    out_d = nc.dram_tensor("out", [1, EL], f32, kind="ExternalOutput")

    _swq = [0]

    def _rr(inst):
        q = _swq[0] % 4
        _swq[0] += 1
        if q:
            inst.ins.queue = f"qPoolDynamic{q}"
        return inst

    with TileContext(nc) as tc:
        with (
            tc.tile_pool(name="const", bufs=1) as pK,
            tc.tile_pool(name="h8p", bufs=1) as pH,
            tc.tile_pool(name="adj", bufs=3) as pAdj,
            tc.tile_pool(name="cn", bufs=2) as pCn,
            tc.tile_pool(name="cnT", bufs=NB - 4) as pT,
            tc.tile_pool(name="xcnT", bufs=1) as pXT,
            tc.tile_pool(name="prod", bufs=2) as pPr,
            tc.tile_pool(name="xij", bufs=1) as pXi,
            tc.tile_pool(name="edge", bufs=1) as pC,
        ):
            # ---- constants (f32/i32 first: they must finish before the
            # first 2-byte xbar transpose is in flight) ----
            idx_sb = pK.tile([P, 2 * NB], i32, tag="idx_sb", name="idx_sb")
            nc.sync.dma_start(
                out=idx_sb[:].rearrange("p (t b) -> p t b", t=2),
                in_=idx_d[:, :].rearrange("t (b p) -> p t b", p=P),
            )
            ii = [idx_sb[:, b:b + 1] for b in range(NB)]
            jj = [idx_sb[:, NB + b:NB + b + 1] for b in range(NB)]

            fpk = pK.tile([P, 2 * len(bnames) + 2], f32, tag="fpk",
                          name="fpk")
            nc.sync.dma_start(out=fpk[:], in_=fpk_d[:, :])
            b_sb = {}
            for q, n in enumerate(bnames):
                b_sb[n] = fpk[:, 2 * q:2 * q + 2]
            beta_sb = fpk[:, 10:11]
            lb2_sb = fpk[:, 11:12]

            wa8 = pK.tile([P, 4 * C], fp8, tag="wa8", name="wa8")
            nc.sync.dma_start(out=wa8[:], in_=wa8_d[:, :])
            w1_sb = wa8[:, 0:2 * C]
            w2_sb = wa8[:, 2 * C:4 * C]
            onesb2 = pK.tile([1, P + C], fp8, tag="onesb2", name="onesb2")
            nc.sync.dma_start(out=onesb2[:], in_=onesb2_d[:, :])
            ones_sb = onesb2[:, 0:P]
            b2row_sb = onesb2[:, P:P + C]

            wc_t = pK.tile([P, 8 * C], bf16, tag="wc", name="wc")
            nc.sync.dma_start(out=wc_t[:], in_=wc_d[:, :])
            wC_sb = {}
            for q, n in enumerate(("xcn_w1", "xcn_w2", "xij_w", "lin_w1")):
                wC_sb[n] = [wc_t[:, q * 2 * C:q * 2 * C + C],
                            wc_t[:, q * 2 * C + C:(q + 1) * 2 * C]]
            lw2_t = pK.tile([P, 2], bf16, tag="lin_w2", name="lin_w2t")
            nc.sync.dma_start(
                out=lw2_t[:].rearrange("p (k o) -> p k o", k=2),
                in_=lin_w2_d[:, :].rearrange("(k p) o -> p k o", p=P),
            )
            lw2_sb = [lw2_t[:, 0:1], lw2_t[:, 1:2]]

            # stage-A input slabs, loaded in 4 chunks so the first
            # adjacency gathers interleave on the serial DMA resource
            xa8 = pK.tile([P, 2 * N], fp8, tag="xa8", name="xa8")
            xr8t = pK.tile([P, 2 * N], fp8, tag="xr8t", name="xr8t")
            NCK = 4
            for ck in range(NCK):
                W2N = 2 * N // NCK
                # xa8 is [p, (k 2, n N)]: load k-halves of each node range
                for k in range(2):
                    sl = slice(k * N + ck * (N // NCK),
                               k * N + (ck + 1) * (N // NCK))
                    nc.sync.dma_start(out=xa8[:, sl], in_=xa8_d[:, sl])
                sl = slice(ck * W2N, (ck + 1) * W2N)
                nc.sync.dma_start(out=xr8t[:, sl], in_=xr8_d[:, sl])

            out_row = pK.tile([1, EL], f32, tag="out_row", name="out_row")

            # h8[p, T*256 + c] = h[node 128*T + p, channel c] in fp8.
            # DoubleRow stationary slice (chunk, ch): [p][j: stride 256]
            # [c2: 128 contiguous] at offset chunk*512 + ch*128.
            h8 = pH.tile([P, 2 * N], fp8, tag="h8", name="h8")
            h8_v = h8[:].rearrange(
                "p (ck j ch c2) -> p ck ch j c2", ck=NCHUNK, j=2, ch=2)

            # ---- stage B state ----
            cnT = [None] * NB
            xcnT_sb = [
                pXT.tile([P, EL], bf16, tag=f"xcnT{ch}", name=f"xcnT{ch}")
                for ch in range(2)
            ]

            def b_gather(b, which):
                t = pAdj.tile([P, N], fp8, tag=f"a{which}", name=f"a{which}{b}")
                off = (ii if which == "i" else jj)[b]
                _rr(nc.gpsimd.indirect_dma_start(
                    out=t[:], out_offset=None, in_=adjs_d[:, :],
                    in_offset=bass.IndirectOffsetOnAxis(ap=off[:, :1], axis=0),
                ))
                return t

            NP_AND = 8  # AND pieces per block (512 u16 cols each)
            HP = NP_AND // 2
            cn8_map = {}

            def b_and_piece(b, q, ai, aj):
                # per-half cn8 tiles: [128, 2048] u16, transposed as soon as
                # the half's 4 AND pieces are done
                half = q // HP
                if q % HP == 0:
                    cn8_map[(b, half)] = pCn.tile(
                        [P, N // 4], u16, tag="cn8", name=f"cn8_{b}_{half}")
                    if half == 0:
                        cnT[b] = pT.tile([P, N // 2], u16, tag="cnT",
                                         name=f"cnT{b}")
                W = (N // 4) // HP
                sl = slice((q % HP) * W, (q % HP + 1) * W)
                base = half * (N // 4)
                nc.vector.tensor_tensor(
                    out=cn8_map[(b, half)][:, sl],
                    in0=ai[:].bitcast(u16)[:, base + sl.start:base + sl.stop],
                    in1=aj[:].bitcast(u16)[:, base + sl.start:base + sl.stop],
                    op=AND,
                )
                if q % HP == HP - 1:
                    cn8 = cn8_map.pop((b, half))
                    nc.sync.dma_start_transpose(
                        out=cnT[b][:, half * (N // 4):(half + 1) * (N // 4)]
                        .rearrange("p (cl e) -> p cl e", e=P),
                        in_=cn8[:],
                    )

            def b_matmuls(b, psT):
                rhs_v = cnT[b][:].bitcast(fp8).rearrange(
                    "p (ck e j) -> p ck j e", ck=NCHUNK, j=2)
                for ch in range(2):
                    o = psT[ch][:, b * P:(b + 1) * P]
                    for chunk in range(NCHUNK):
                        nc.tensor.matmul(
                            o, h8_v[:, chunk, ch, :, :], rhs_v[:, chunk, :, :],
                            start=(chunk == 0), stop=(chunk == NCHUNK - 1),
                            perf_mode=DR,
                        )

            def b_copyout(b, psT):
                for ch in range(2):
                    nc.vector.tensor_copy(
                        xcnT_sb[ch][:, b * P:(b + 1) * P],
                        psT[ch][:, b * P:(b + 1) * P],
                    )

            # ---- stage C (transposed-layout edge MLPs, bf16) ----
            prodT_map = {}
            xi_map = {}

            def c_gathers(b):
                xi = pXi.tile([P, C], bf16, tag=f"xi{b}", name=f"xi{b}")
                _rr(nc.gpsimd.indirect_dma_start(
                    out=xi[:], out_offset=None, in_=x_d[:, :],
                    in_offset=bass.IndirectOffsetOnAxis(
                        ap=ii[b][:, :1], axis=0),
                ))
                xj = pXi.tile([P, C], bf16, tag=f"xj{b}", name=f"xj{b}")
                _rr(nc.gpsimd.indirect_dma_start(
                    out=xj[:], out_offset=None, in_=x_d[:, :],
                    in_offset=bass.IndirectOffsetOnAxis(
                        ap=jj[b][:, :1], axis=0),
                ))
                xi_map[b] = (xi, xj)

            def stage_c_prod(grp):
                W = CGRP * P
                prodT = pPr.tile([P, 2 * W], bf16, tag="prodT", name=f"prodT{grp}")
                prodT_v = prodT[:].rearrange(
                    "p (blk hh e) -> p blk hh e", blk=CGRP, e=P)
                prodT_map[grp] = prodT
                for t2, b in enumerate(range(grp * CGRP, (grp + 1) * CGRP)):
                    xi, xj = xi_map[b]
                    pt = pPr.tile([P, C], bf16, tag="prod", name=f"prod{b}")
                    nc.vector.tensor_tensor(
                        out=pt[:], in0=xi[:], in1=xj[:], op=MUL
                    )
                    nc.sync.dma_start_transpose(
                        out=prodT_v[:, t2, :, :], in_=pt[:],
                    )

            def stage_c(grp, psC, psO):
                W = CGRP * P  # 512 edges

                def mlp_layer(pair, wname, bname, outtag, packed=False):
                    outs = []
                    for h in range(2):
                        ps = psC.tile([P, W], f32, tag="psc",
                                      name=f"psc_{grp}_{outtag}{h}")
                        if packed:
                            rhs_v = pair[:].rearrange(
                                "p (blk hh e) -> p blk hh e", blk=CGRP, e=P)
                            r0, r1 = rhs_v[:, :, 0, :], rhs_v[:, :, 1, :]
                        else:
                            r0, r1 = pair
                        nc.tensor.matmul(
                            ps[:], wC_sb[wname][0][:, h * P:(h + 1) * P],
                            r0, start=True, stop=False,
                        )
                        nc.tensor.matmul(
                            ps[:], wC_sb[wname][1][:, h * P:(h + 1) * P],
                            r1, start=False, stop=True,
                        )
                        t = pC.tile([P, W], bf16, tag=f"{outtag}{h}",
                                    name=f"{outtag}{h}_{grp}")
                        nc.scalar.activation(
                            t[:], ps[:], Relu, bias=b_sb[bname][:, h:h + 1]
                        )
                        outs.append(t)
                    return outs

                xijT = mlp_layer(prodT_map[grp], "xij_w", "xij_b", "xijT",
                                 packed=True)
                xcn_pair = (xcnT_sb[0][:, grp * W:(grp + 1) * W],
                            xcnT_sb[1][:, grp * W:(grp + 1) * W])
                u1T = mlp_layer(xcn_pair, "xcn_w1", "xcn_b1", "u1T")
                u2T = mlp_layer([u1T[0][:], u1T[1][:]], "xcn_w2", "xcn_b2",
                                "u2T")
                zT = []
                for h in range(2):
                    zb = pC.tile([P, W], bf16, tag=f"zb{h}", name=f"zb{h}_{grp}")
                    nc.vector.tensor_tensor(
                        out=zb[:], in0=u2T[h][:],
                        in1=beta_sb[:, 0:1].to_broadcast([P, W]), op=MUL,
                    )
                    zt = pC.tile([P, W], bf16, tag=f"zT{h}", name=f"zT{h}_{grp}")
                    nc.vector.tensor_tensor(
                        out=zt[:], in0=zb[:], in1=xijT[h][:], op=ADD
                    )
                    zT.append(zt)
                vT = mlp_layer([zT[0][:], zT[1][:]], "lin_w1", "lin_b1", "vT")
                pso = psO.tile([1, W], f32, tag="pso", name=f"pso{grp}")
                nc.tensor.matmul(
                    pso[:], lw2_sb[0][:], vT[0][:], start=True, stop=False
                )
                nc.tensor.matmul(
                    pso[:], lw2_sb[1][:], vT[1][:], start=False, stop=True
                )
                nc.scalar.activation(
                    out_row[0:1, grp * W:(grp + 1) * W], pso[:],
                    Ident, bias=lb2_sb[0:1, 0:1],
                )

            # ---- emission ----
            if True:
                gathered = {}
                with tc.tile_pool(name="stA", bufs=3) as pA, \
                     tc.tile_pool(name="relp", bufs=16) as pRel, \
                     tc.tile_pool(name="psA", bufs=4, space="PSUM") as psA, \
                     tc.tile_pool(name="psL2", bufs=4, space="PSUM") as psL2:
                    and_q = []
                    for g in range(NG):
                        if g % 2 == 1:
                            c_gathers(g // 2)
                        if g % 2 == 0:
                            b = g // 2
                            gathered[b] = (b_gather(b, "i"),
                                           b_gather(b, "j"))
                            for q in range(NP_AND):
                                and_q.append((b, q))

                        # AND pieces first: they track the gathers; the
                        # adds behind them can lag (rel pool absorbs it)
                        navail = (g // 2 + 1) * NP_AND
                        emitted = NP_AND * NB - len(and_q)
                        budget = NP_AND
                        while and_q and budget > 0 and emitted < navail:
                            b2, q2 = and_q.pop(0)
                            b_and_piece(b2, q2, *gathered[b2])
                            emitted += 1
                            budget -= 1

                        m0 = g * AGRP
                        y1T = []
                        for ch in range(2):
                            ps = psA.tile([P, AGRP], f32, tag="psA",
                                          name=f"psA_{g}{ch}")
                            nc.tensor.matmul(
                                ps[:], w1_sb[:, ch * P:(ch + 1) * P],
                                xa8[:, m0:m0 + AGRP],
                                start=True, stop=False,
                            )
                            nc.tensor.matmul(
                                ps[:], w1_sb[:, C + ch * P:C + (ch + 1) * P],
                                xa8[:, N + m0:N + m0 + AGRP],
                                start=False, stop=True,
                            )
                            t = pA.tile([P, AGRP], fp8, tag=f"y1T{ch}",
                                        name=f"y1T{ch}_{g}")
                            nc.scalar.activation(
                                t[:], ps[:], Relu,
                                bias=b_sb["xlin_b1"][:, ch:ch + 1],
                            )
                            y1T.append(t)
                        for t2 in range(4):
                            T = 4 * g + t2
                            ps2 = psL2.tile([P, C], f32, tag="psL2",
                                            name=f"psL2_{T}")
                            nc.tensor.matmul(
                                ps2[:], y1T[0][:, t2 * P:(t2 + 1) * P],
                                w2_sb[:, 0:C], start=True, stop=False,
                            )
                            nc.tensor.matmul(
                                ps2[:], y1T[1][:, t2 * P:(t2 + 1) * P],
                                w2_sb[:, C:2 * C], start=False, stop=False,
                            )
                            nc.tensor.matmul(
                                ps2[:], ones_sb[0:1, :], b2row_sb[0:1, :],
                                start=False, stop=True,
                            )
                            rel = pRel.tile([P, C], bf16, tag="rel",
                                            name=f"rel_{T}")
                            nc.vector.tensor_scalar(
                                rel[:], ps2[:], 0.0, None,
                                mybir.AluOpType.max)
                            nc.vector.tensor_tensor(
                                out=h8[:, T * C:(T + 1) * C],
                                in0=xr8t[:, T * C:(T + 1) * C],
                                in1=rel[:], op=ADD,
                            )
                    while and_q:
                        b2, q2 = and_q.pop(0)
                        b_and_piece(b2, q2, *gathered[b2])

                with tc.tile_pool(name="psB", bufs=1, space="PSUM") as psBp, \
                     tc.tile_pool(name="psC", bufs=2, space="PSUM") as psC, \
                     tc.tile_pool(name="psO", bufs=1, space="PSUM") as psO:
                    psT = [
                        psBp.tile([P, EL], f32, tag=f"psT{ch}",
                                  name=f"psT{ch}")
                        for ch in range(2)
                    ]
                    for b in range(NB):
                        if b == 0:
                            stage_c_prod(0)
                        if b == 2:
                            stage_c_prod(1)
                        b_matmuls(b, psT)
                        b_copyout(b, psT)
                        if b == CGRP - 1:
                            stage_c(0, psC, psO)
                        if b == NB - 1:
                            stage_c(1, psC, psO)

            nc.sync.dma_start(out=out_d[:, :], in_=out_row[0:1, :])

    return _split_multi_waits(nc) if split_waits else nc


def _col_shuffle_perm():
    """d[m]: DRAM column position for original node m so the cnT transpose
    pairing (k = 2p + j) matches h8's block-major slots (node 128T + p at
    chunk T//2, j = T%2)."""
    m = np.arange(N)
    T = m // P
    p = m % P
    return 256 * (T // 2) + 2 * p + (T % 2)


def kernel(**inputs):
    from concourse.bass_utils import run_bass_kernel_spmd

    if "nc" not in _CACHE:
        _CACHE["nc"] = _build()
    nc = _CACHE["nc"]

    x = np.ascontiguousarray(inputs["x"], dtype=np.float32)
    adj8 = np.ascontiguousarray(inputs["adj"]).astype(ml_dtypes.float8_e4m3)
    d = _col_shuffle_perm()
    adjs = np.empty_like(adj8)
    adjs[:, d] = adj8
    tar = np.asarray(inputs["tar_ei"]).astype(np.int32)

    x8 = x.astype(ml_dtypes.float8_e4m3)
    # xa8[p, k*N + n] = x[n, k*128 + p]
    xa8 = np.ascontiguousarray(
        x8.T.reshape(2, P, N).transpose(1, 0, 2).reshape(P, 2 * N))
    # xr8t[p, T*C + c] = x[T*128 + p, c]
    xr8t = np.ascontiguousarray(
        x8.reshape(NT, P, C).transpose(1, 0, 2).reshape(P, NT * C))

    def wtile(w, dt):
        # [p, (ksub 2, cout C)] from [C, C]
        return np.ascontiguousarray(
            np.asarray(w).astype(dt).reshape(2, P, C).transpose(1, 0, 2)
            .reshape(P, 2 * C))

    wa8 = np.concatenate(
        [wtile(inputs["xlin_w1"], ml_dtypes.float8_e4m3),
         wtile(inputs["xlin_w2"], ml_dtypes.float8_e4m3)], axis=1)
    onesb2 = np.concatenate(
        [np.ones((1, P), np.float32),
         np.asarray(inputs["xlin_b2"], np.float32).reshape(1, C)],
        axis=1).astype(ml_dtypes.float8_e4m3)
    wc = np.concatenate(
        [wtile(inputs[n], ml_dtypes.bfloat16)
         for n in ("xcn_w1", "xcn_w2", "xij_w", "lin_w1")], axis=1)

    def btile(b):
        return np.ascontiguousarray(
            np.asarray(b, dtype=np.float32).reshape(2, P).T)

    fpk = np.concatenate(
        [btile(inputs[n]) for n in
         ("xlin_b1", "xcn_b1", "xcn_b2", "xij_b", "lin_b1")] +
        [np.full((P, 1), np.asarray(inputs["beta"]).reshape(-1)[0],
                 dtype=np.float32),
         np.full((P, 1), np.asarray(inputs["lin_b2"]).reshape(-1)[0],
                 dtype=np.float32)],
        axis=1)

    common = {
        "x": x.astype(ml_dtypes.bfloat16),
        "xa8": xa8,
        "xr8t": xr8t,
        "adjs": adjs,
        "wa8": wa8,
        "onesb2": onesb2,
        "wc": wc,
        "fpk": fpk,
        "lin_w2": np.ascontiguousarray(inputs["lin_w2"]).astype(
            ml_dtypes.bfloat16),
    }

    in_maps = []
    for c in range(NCORES):
        m = dict(common)
        m["idx"] = np.ascontiguousarray(tar[:, c * EL:(c + 1) * EL])
        in_maps.append(m)

    res = run_bass_kernel_spmd(
        nc, in_maps, core_ids=list(range(NCORES)), trace=TRACE
    )
    global LAST_RESULT
    LAST_RESULT = res
    out = np.concatenate(
        [res.results[c]["out"].reshape(EL, 1) for c in range(NCORES)], axis=0
    )
    return out.astype(np.float32)


# revision 48
# speedup vs baseline: 1.8443x; 1.0255x over previous
"""CNLinkPredictor Trainium2 kernel (fp8 DoubleRow common-neighbor pipeline).

Edge-sharded across 8 NeuronCores (1024 target edges each); x, adj, and the
MLP weights are replicated. Per core:

  A) h = x + MLP(x), finishing in NATURAL (node-partition) layout so the
     result lands directly in the fp8 block-major layout the DoubleRow
     matmul wants for its stationary operand:
       - L1 stays transposed: y1T = relu(W1^T xT) (fp8 weights + moving).
       - L2 flips orientation per 128-node tile: psum[node, c] accumulates
         y1T-chunks as stationary with W2 moving; the bias lands via a K=1
         ones-row x b2-row matmul; relu on ACT; DVE adds the x residual and
         writes fp8 straight into h8[p, T*256:(T+1)*256].
  B) per 128-edge block: one indirect full-row gather per endpoint from a
     column-shuffled adjacency (host prep), uint16 bitwise-AND (exact for
     0/1 fp8 patterns, runs at the 2-byte DVE rate), two uint16 packed
     transposes (half the xbar tile count of bf16), then 64 DoubleRow fp8
     matmuls accumulating xcnT[c, e] directly - the adjacency column
     shuffle makes the transpose pairing k=2p+j line up with h8's block
     slots.
  C) edge MLPs in transposed layout (bf16): xcnT comes straight out of B's
     PSUM (no transpose), xi*xj is gathered/multiplied/transposed as before.

Hardware pitfalls this kernel works around (carried from the previous
session, all re-validated):
  - walrus accepts at most ONE sync-wait per instruction
    (_apply_tile_patch + _split_multi_waits).
  - Concurrent 4-byte DMA traffic corrupts in-flight 2-byte xbar
    DMA-transposes: every steady-state transfer is <= 2 bytes/element;
    f32/i32 loads happen up front, the single f32 store happens last.
  - xbar transposes need contiguous per-partition destinations.
  - DoubleRow needs a block-major stationary operand (pair step % 16 == 0);
    the byte-interleaved transpose output is only legal as the MOVING
    operand (verified empirically - the ISA check rejects it as weights).
  - PSUM zero regions are 2048 B: accumulation groups sharing a psum tile
    run strictly block-major so a start=True never clobbers a neighbor.
"""

import numpy as np
import ml_dtypes

N = 8192
C = 256
E = 8192
NCORES = 8
EL = E // NCORES          # edges per core
P = 128
NB = EL // P              # edge blocks per core (8)
NCHUNK = N // 256         # 256-node DoubleRow chunks (32)
NT = N // P               # stage-A node tiles (64)
AGRP = 512                # stage-A node group (4 tiles)
NG = N // AGRP            # stage-A groups (16)
CGRP = 4                  # stage-C blocks per group (512 edges)

_CACHE = {}
TRACE = False
LAST_RESULT = None


def _apply_tile_patch():
    """Split the Tile tail-drain's multi-sem wait onto individual SP nops."""
    from concourse.tile import TileContext
    from concourse.vector_clock import ScopedClock

    if getattr(TileContext, "_drain_patched", False):
        return

    def _patched(self, tick_clock, wait_clock):
        nc = self.nc
        collector = nc.sync.nop()
        wait_clock.add_sem_waits(
            collector.ins, ScopedClock({None: tick_clock.global_clock})
        )
        si = collector.ins.sync_info
        waits = list(si.on_wait) if si is not None and si.on_wait else []
        if si is not None and len(waits) > 1:
            name_to_handle = {h.name: h for h in self.sems.allocated().values()}
            si.on_wait = [waits[0]]
            for w in waits[1:]:
                op = {
                    "sem-ge-imm": "sem-ge",
                    "sem-eq-imm": "sem-eq",
                    "sem-le-imm": "sem-le",
                }.get(str(w.wait_mode), "sem-ge")
                nc.sync.nop().wait_op(name_to_handle[w.ant_name], w.wait_value, op)
        nc.sync.drain()
        nc.all_engine_barrier()
        assert self.sems is not None
        popped = nc._tile_sem_poison_stack.pop()
        assert popped is self._sem_poison
        nc.clear_and_free_semaphores(list(self.sems.allocated().values()))
        nc.all_engine_barrier()

    TileContext._drain_and_barrier = _patched
    TileContext._drain_patched = True


def _split_multi_waits(nc):
    """Hoist extra sync-waits onto same-engine NoOps (sequential waits ==
    ANDed waits); this walrus build allows one wait per instruction."""
    import concourse.mybir as mybir

    cnt = 0
    for fn in nc.m.functions:
        for bb in fn.blocks:
            out = []
            for inst in bb.instructions:
                si = getattr(inst, "sync_info", None)
                waits = list(si.on_wait) if si is not None and si.on_wait else []
                if len(waits) > 1:
                    for w in waits[:-1]:
                        nop = mybir.InstNoOp(name=f"ws-{cnt}", ins=[], outs=[])
                        cnt += 1
                        nop.engine = inst.engine
                        nop.sync_info = mybir.SyncInfo(on_wait=[w], on_update=[])
                        out.append(nop)
                    si.on_wait = [waits[-1]]
                out.append(inst)
            bb.instructions = out
    return nc


def _build(split_waits=True):
    import concourse.bass as bass
    import concourse.mybir as mybir
    from concourse.tile import TileContext

    _apply_tile_patch()

    f32 = mybir.dt.float32
    bf16 = mybir.dt.bfloat16
    fp8 = mybir.dt.float8e4
    u16 = mybir.dt.uint16
    i32 = mybir.dt.int32
    Relu = mybir.ActivationFunctionType.Relu
    Ident = mybir.ActivationFunctionType.Identity
    MUL = mybir.AluOpType.mult
    ADD = mybir.AluOpType.add
    AND = mybir.AluOpType.bitwise_and
    DR = mybir.MatmulPerfMode.DoubleRow

    nc = bass.Bass(num_swdge_queues=4, dynamic_dma_scratch_size=32768)

    # host-pretiled: xa8[p, k*N + n] = x[n, k*128 + p] (fp8)
    xa8_d = nc.dram_tensor("xa8", [P, 2 * N], fp8, kind="ExternalInput")
    # host-pretiled: xr8[p, T*C + c] = x[T*128 + p, c] (fp8)
    xr8_d = nc.dram_tensor("xr8t", [P, 2 * N], fp8, kind="ExternalInput")
    x_d = nc.dram_tensor("x", [N, C], bf16, kind="ExternalInput")
    adjs_d = nc.dram_tensor("adjs", [N, N], fp8, kind="ExternalInput")
    idx_d = nc.dram_tensor("idx", [2, EL], i32, kind="ExternalInput")
    # host-packed fp8 stage-A weights: [p, (which 2, k 2, cout 256)] + ones/b2
    wa8_d = nc.dram_tensor("wa8", [P, 4 * C], fp8, kind="ExternalInput")
    onesb2_d = nc.dram_tensor("onesb2", [1, P + C], fp8, kind="ExternalInput")
    # host-packed bf16 stage-C weights: [p, (which 4, k 2, cout 256)]
    wc_d = nc.dram_tensor("wc", [P, 8 * C], bf16, kind="ExternalInput")
    lin_w2_d = nc.dram_tensor("lin_w2", [C, 1], bf16, kind="ExternalInput")
    bnames = ["xlin_b1", "xcn_b1", "xcn_b2", "xij_b", "lin_b1"]
    # host-packed f32: [p, (bias pairs 10, beta 1, lin_b2 1)]
    fpk_d = nc.dram_tensor("fpk", [P, 2 * len(bnames) + 2], f32,
                           kind="ExternalInput")
    out_d = nc.dram_tensor("out", [1, EL], f32, kind="ExternalOutput")

    _swq = [0]

    def _rr(inst):
        q = _swq[0] % 4
        _swq[0] += 1
        if q:
            inst.ins.queue = f"qPoolDynamic{q}"
        return inst

    with TileContext(nc) as tc:
        with (
            tc.tile_pool(name="const", bufs=1) as pK,
            tc.tile_pool(name="h8p", bufs=1) as pH,
            tc.tile_pool(name="adj", bufs=3) as pAdj,
            tc.tile_pool(name="cn", bufs=2) as pCn,
            tc.tile_pool(name="cnT", bufs=NB - 4) as pT,
            tc.tile_pool(name="xcnT", bufs=1) as pXT,
            tc.tile_pool(name="prod", bufs=2) as pPr,
            tc.tile_pool(name="xij", bufs=1) as pXi,
            tc.tile_pool(name="edge", bufs=1) as pC,
        ):
            # ---- constants (f32/i32 first: they must finish before the
            # first 2-byte xbar transpose is in flight) ----
            idx_sb = pK.tile([P, 2 * NB], i32, tag="idx_sb", name="idx_sb")
            nc.sync.dma_start(
                out=idx_sb[:].rearrange("p (t b) -> p t b", t=2),
                in_=idx_d[:, :].rearrange("t (b p) -> p t b", p=P),
            )
            ii = [idx_sb[:, b:b + 1] for b in range(NB)]
            jj = [idx_sb[:, NB + b:NB + b + 1] for b in range(NB)]

            fpk = pK.tile([P, 2 * len(bnames) + 2], f32, tag="fpk",
                          name="fpk")
            nc.sync.dma_start(out=fpk[:], in_=fpk_d[:, :])
            b_sb = {}
            for q, n in enumerate(bnames):
                b_sb[n] = fpk[:, 2 * q:2 * q + 2]
            beta_sb = fpk[:, 10:11]
            lb2_sb = fpk[:, 11:12]

            wa8 = pK.tile([P, 4 * C], fp8, tag="wa8", name="wa8")
            nc.sync.dma_start(out=wa8[:], in_=wa8_d[:, :])
            w1_sb = wa8[:, 0:2 * C]
            w2_sb = wa8[:, 2 * C:4 * C]
            onesb2 = pK.tile([1, P + C], fp8, tag="onesb2", name="onesb2")
            nc.sync.dma_start(out=onesb2[:], in_=onesb2_d[:, :])
            ones_sb = onesb2[:, 0:P]
            b2row_sb = onesb2[:, P:P + C]

            wc_t = pK.tile([P, 8 * C], bf16, tag="wc", name="wc")
            nc.sync.dma_start(out=wc_t[:], in_=wc_d[:, :])
            wC_sb = {}
            for q, n in enumerate(("xcn_w1", "xcn_w2", "xij_w", "lin_w1")):
                wC_sb[n] = [wc_t[:, q * 2 * C:q * 2 * C + C],
                            wc_t[:, q * 2 * C + C:(q + 1) * 2 * C]]
            lw2_t = pK.tile([P, 2], bf16, tag="lin_w2", name="lin_w2t")
            nc.sync.dma_start(
                out=lw2_t[:].rearrange("p (k o) -> p k o", k=2),
                in_=lin_w2_d[:, :].rearrange("(k p) o -> p k o", p=P),
            )
            lw2_sb = [lw2_t[:, 0:1], lw2_t[:, 1:2]]

            # stage-A input slabs, loaded in 4 chunks so the first
            # adjacency gathers interleave on the serial DMA resource
            xa8 = pK.tile([P, 2 * N], fp8, tag="xa8", name="xa8")
            xr8t = pK.tile([P, 2 * N], fp8, tag="xr8t", name="xr8t")
            NCK = 4
            for ck in range(NCK):
                W2N = 2 * N // NCK
                # xa8 is [p, (k 2, n N)]: load k-halves of each node range
                for k in range(2):
                    sl = slice(k * N + ck * (N // NCK),
                               k * N + (ck + 1) * (N // NCK))
                    nc.sync.dma_start(out=xa8[:, sl], in_=xa8_d[:, sl])
                sl = slice(ck * W2N, (ck + 1) * W2N)
                nc.sync.dma_start(out=xr8t[:, sl], in_=xr8_d[:, sl])

            out_row = pK.tile([1, EL], f32, tag="out_row", name="out_row")

            # h8[p, T*256 + c] = h[node 128*T + p, channel c] in fp8.
            # DoubleRow stationary slice (chunk, ch): [p][j: stride 256]
            # [c2: 128 contiguous] at offset chunk*512 + ch*128.
            h8 = pH.tile([P, 2 * N], fp8, tag="h8", name="h8")
            h8_v = h8[:].rearrange(
                "p (ck j ch c2) -> p ck ch j c2", ck=NCHUNK, j=2, ch=2)

            # ---- stage B state ----
            cnT = [None] * NB
            xcnT_sb = [
                pXT.tile([P, EL], bf16, tag=f"xcnT{ch}", name=f"xcnT{ch}")
                for ch in range(2)
            ]

            def b_gather(b, which):
                t = pAdj.tile([P, N], fp8, tag=f"a{which}", name=f"a{which}{b}")
                off = (ii if which == "i" else jj)[b]
                _rr(nc.gpsimd.indirect_dma_start(
                    out=t[:], out_offset=None, in_=adjs_d[:, :],
                    in_offset=bass.IndirectOffsetOnAxis(ap=off[:, :1], axis=0),
                ))
                return t

            NP_AND = 16  # AND pieces per block (256 u16 cols each)
            HP = NP_AND // 2
            cn8_map = {}

            def b_and_piece(b, q, ai, aj):
                # per-half cn8 tiles: [128, 2048] u16, transposed as soon as
                # the half's 4 AND pieces are done
                half = q // HP
                if q % HP == 0:
                    cn8_map[(b, half)] = pCn.tile(
                        [P, N // 4], u16, tag="cn8", name=f"cn8_{b}_{half}")
                    if half == 0:
                        cnT[b] = pT.tile([P, N // 2], u16, tag="cnT",
                                         name=f"cnT{b}")
                W = (N // 4) // HP
                sl = slice((q % HP) * W, (q % HP + 1) * W)
                base = half * (N // 4)
                nc.vector.tensor_tensor(
                    out=cn8_map[(b, half)][:, sl],
                    in0=ai[:].bitcast(u16)[:, base + sl.start:base + sl.stop],
                    in1=aj[:].bitcast(u16)[:, base + sl.start:base + sl.stop],
                    op=AND,
                )
                if q % HP == HP - 1:
                    cn8 = cn8_map.pop((b, half))
                    nc.sync.dma_start_transpose(
                        out=cnT[b][:, half * (N // 4):(half + 1) * (N // 4)]
                        .rearrange("p (cl e) -> p cl e", e=P),
                        in_=cn8[:],
                    )

            def b_matmuls(b, psT):
                rhs_v = cnT[b][:].bitcast(fp8).rearrange(
                    "p (ck e j) -> p ck j e", ck=NCHUNK, j=2)
                for ch in range(2):
                    o = psT[ch][:, b * P:(b + 1) * P]
                    for chunk in range(NCHUNK):
                        nc.tensor.matmul(
                            o, h8_v[:, chunk, ch, :, :], rhs_v[:, chunk, :, :],
                            start=(chunk == 0), stop=(chunk == NCHUNK - 1),
                            perf_mode=DR,
                        )

            def b_copyout(b, psT):
                for ch in range(2):
                    nc.vector.tensor_copy(
                        xcnT_sb[ch][:, b * P:(b + 1) * P],
                        psT[ch][:, b * P:(b + 1) * P],
                    )

            # ---- stage C (transposed-layout edge MLPs, bf16) ----
            prodT_map = {}
            xi_map = {}

            def c_gathers(b):
                xi = pXi.tile([P, C], bf16, tag=f"xi{b}", name=f"xi{b}")
                _rr(nc.gpsimd.indirect_dma_start(
                    out=xi[:], out_offset=None, in_=x_d[:, :],
                    in_offset=bass.IndirectOffsetOnAxis(
                        ap=ii[b][:, :1], axis=0),
                ))
                xj = pXi.tile([P, C], bf16, tag=f"xj{b}", name=f"xj{b}")
                _rr(nc.gpsimd.indirect_dma_start(
                    out=xj[:], out_offset=None, in_=x_d[:, :],
                    in_offset=bass.IndirectOffsetOnAxis(
                        ap=jj[b][:, :1], axis=0),
                ))
                xi_map[b] = (xi, xj)

            def stage_c_prod(grp):
                W = CGRP * P
                prodT = pPr.tile([P, 2 * W], bf16, tag="prodT", name=f"prodT{grp}")
                prodT_v = prodT[:].rearrange(
                    "p (blk hh e) -> p blk hh e", blk=CGRP, e=P)
                prodT_map[grp] = prodT
                for t2, b in enumerate(range(grp * CGRP, (grp + 1) * CGRP)):
                    xi, xj = xi_map[b]
                    pt = pPr.tile([P, C], bf16, tag="prod", name=f"prod{b}")
                    nc.vector.tensor_tensor(
                        out=pt[:], in0=xi[:], in1=xj[:], op=MUL
                    )
                    nc.sync.dma_start_transpose(
                        out=prodT_v[:, t2, :, :], in_=pt[:],
                    )

            def stage_c(grp, psC, psO):
                W = CGRP * P  # 512 edges

                def mlp_layer(pair, wname, bname, outtag, packed=False):
                    outs = []
                    for h in range(2):
                        ps = psC.tile([P, W], f32, tag="psc",
                                      name=f"psc_{grp}_{outtag}{h}")
                        if packed:
                            rhs_v = pair[:].rearrange(
                                "p (blk hh e) -> p blk hh e", blk=CGRP, e=P)
                            r0, r1 = rhs_v[:, :, 0, :], rhs_v[:, :, 1, :]
                        else:
                            r0, r1 = pair
                        nc.tensor.matmul(
                            ps[:], wC_sb[wname][0][:, h * P:(h + 1) * P],
                            r0, start=True, stop=False,
                        )
                        nc.tensor.matmul(
                            ps[:], wC_sb[wname][1][:, h * P:(h + 1) * P],
                            r1, start=False, stop=True,
                        )
                        t = pC.tile([P, W], bf16, tag=f"{outtag}{h}",
                                    name=f"{outtag}{h}_{grp}")
                        nc.scalar.activation(
                            t[:], ps[:], Relu, bias=b_sb[bname][:, h:h + 1]
                        )
                        outs.append(t)
                    return outs

                xijT = mlp_layer(prodT_map[grp], "xij_w", "xij_b", "xijT",
                                 packed=True)
                xcn_pair = (xcnT_sb[0][:, grp * W:(grp + 1) * W],
                            xcnT_sb[1][:, grp * W:(grp + 1) * W])
                u1T = mlp_layer(xcn_pair, "xcn_w1", "xcn_b1", "u1T")
                u2T = mlp_layer([u1T[0][:], u1T[1][:]], "xcn_w2", "xcn_b2",
                                "u2T")
                zT = []
                for h in range(2):
                    zb = pC.tile([P, W], bf16, tag=f"zb{h}", name=f"zb{h}_{grp}")
                    nc.vector.tensor_tensor(
                        out=zb[:], in0=u2T[h][:],
                        in1=beta_sb[:, 0:1].to_broadcast([P, W]), op=MUL,
                    )
                    zt = pC.tile([P, W], bf16, tag=f"zT{h}", name=f"zT{h}_{grp}")
                    nc.vector.tensor_tensor(
                        out=zt[:], in0=zb[:], in1=xijT[h][:], op=ADD
                    )
                    zT.append(zt)
                vT = mlp_layer([zT[0][:], zT[1][:]], "lin_w1", "lin_b1", "vT")
                pso = psO.tile([1, W], f32, tag="pso", name=f"pso{grp}")
                nc.tensor.matmul(
                    pso[:], lw2_sb[0][:], vT[0][:], start=True, stop=False
                )
                nc.tensor.matmul(
                    pso[:], lw2_sb[1][:], vT[1][:], start=False, stop=True
                )
                nc.scalar.activation(
                    out_row[0:1, grp * W:(grp + 1) * W], pso[:],
                    Ident, bias=lb2_sb[0:1, 0:1],
                )

            # ---- emission ----
            if True:
                gathered = {}
                with tc.tile_pool(name="stA", bufs=3) as pA, \
                     tc.tile_pool(name="relp", bufs=16) as pRel, \
                     tc.tile_pool(name="psA", bufs=4, space="PSUM") as psA, \
                     tc.tile_pool(name="psL2", bufs=4, space="PSUM") as psL2:
                    and_q = []
                    for g in range(NG):
                        if g % 2 == 1:
                            c_gathers(g // 2)
                        if g % 2 == 0:
                            b = g // 2
                            gathered[b] = (b_gather(b, "i"),
                                           b_gather(b, "j"))
                            for q in range(NP_AND):
                                and_q.append((b, q))

                        # AND pieces first: they track the gathers; the
                        # adds behind them can lag (rel pool absorbs it)
                        navail = (g // 2 + 1) * NP_AND
                        emitted = NP_AND * NB - len(and_q)
                        budget = NP_AND
                        while and_q and budget > 0 and emitted < navail:
                            b2, q2 = and_q.pop(0)
                            b_and_piece(b2, q2, *gathered[b2])
                            emitted += 1
                            budget -= 1

                        m0 = g * AGRP
                        y1T = []
                        for ch in range(2):
                            ps = psA.tile([P, AGRP], f32, tag="psA",
                                          name=f"psA_{g}{ch}")
                            nc.tensor.matmul(
                                ps[:], w1_sb[:, ch * P:(ch + 1) * P],
                                xa8[:, m0:m0 + AGRP],
                                start=True, stop=False,
                            )
                            nc.tensor.matmul(
                                ps[:], w1_sb[:, C + ch * P:C + (ch + 1) * P],
                                xa8[:, N + m0:N + m0 + AGRP],
                                start=False, stop=True,
                            )
                            t = pA.tile([P, AGRP], fp8, tag=f"y1T{ch}",
                                        name=f"y1T{ch}_{g}")
                            nc.scalar.activation(
                                t[:], ps[:], Relu,
                                bias=b_sb["xlin_b1"][:, ch:ch + 1],
                            )
                            y1T.append(t)
                        for t2 in range(4):
                            T = 4 * g + t2
                            ps2 = psL2.tile([P, C], f32, tag="psL2",
                                            name=f"psL2_{T}")
                            nc.tensor.matmul(
                                ps2[:], y1T[0][:, t2 * P:(t2 + 1) * P],
                                w2_sb[:, 0:C], start=True, stop=False,
                            )
                            nc.tensor.matmul(
                                ps2[:], y1T[1][:, t2 * P:(t2 + 1) * P],
                                w2_sb[:, C:2 * C], start=False, stop=False,
                            )
                            nc.tensor.matmul(
                                ps2[:], ones_sb[0:1, :], b2row_sb[0:1, :],
                                start=False, stop=True,
                            )
                            rel = pRel.tile([P, C], bf16, tag="rel",
                                            name=f"rel_{T}")
                            nc.vector.tensor_scalar(
                                rel[:], ps2[:], 0.0, None,
                                mybir.AluOpType.max)
                            nc.vector.tensor_tensor(
                                out=h8[:, T * C:(T + 1) * C],
                                in0=xr8t[:, T * C:(T + 1) * C],
                                in1=rel[:], op=ADD,
                            )
                    while and_q:
                        b2, q2 = and_q.pop(0)
                        b_and_piece(b2, q2, *gathered[b2])

                with tc.tile_pool(name="psB", bufs=1, space="PSUM") as psBp, \
                     tc.tile_pool(name="psC", bufs=2, space="PSUM") as psC, \
                     tc.tile_pool(name="psO", bufs=1, space="PSUM") as psO:
                    psT = [
                        psBp.tile([P, EL], f32, tag=f"psT{ch}",
                                  name=f"psT{ch}")
                        for ch in range(2)
                    ]
                    for b in range(NB):
                        if b == 0:
                            stage_c_prod(0)
                        if b == 2:
                            stage_c_prod(1)
                        b_matmuls(b, psT)
                        b_copyout(b, psT)
                        if b == CGRP - 1:
                            stage_c(0, psC, psO)
                        if b == NB - 1:
                            stage_c(1, psC, psO)

            nc.sync.dma_start(out=out_d[:, :], in_=out_row[0:1, :])

    return _split_multi_waits(nc) if split_waits else nc


def _col_shuffle_perm():
    """d[m]: DRAM column position for original node m so the cnT transpose
    pairing (k = 2p + j) matches h8's block-major slots (node 128T + p at
    chunk T//2, j = T%2)."""
    m = np.arange(N)
    T = m // P
    p = m % P
    return 256 * (T // 2) + 2 * p + (T % 2)


def kernel(**inputs):
    from concourse.bass_utils import run_bass_kernel_spmd

    if "nc" not in _CACHE:
        _CACHE["nc"] = _build()
    nc = _CACHE["nc"]

    x = np.ascontiguousarray(inputs["x"], dtype=np.float32)
    adj8 = np.ascontiguousarray(inputs["adj"]).astype(ml_dtypes.float8_e4m3)
    d = _col_shuffle_perm()
    adjs = np.empty_like(adj8)
    adjs[:, d] = adj8
    tar = np.asarray(inputs["tar_ei"]).astype(np.int32)

    x8 = x.astype(ml_dtypes.float8_e4m3)
    # xa8[p, k*N + n] = x[n, k*128 + p]
    xa8 = np.ascontiguousarray(
        x8.T.reshape(2, P, N).transpose(1, 0, 2).reshape(P, 2 * N))
    # xr8t[p, T*C + c] = x[T*128 + p, c]
    xr8t = np.ascontiguousarray(
        x8.reshape(NT, P, C).transpose(1, 0, 2).reshape(P, NT * C))

    def wtile(w, dt):
        # [p, (ksub 2, cout C)] from [C, C]
        return np.ascontiguousarray(
            np.asarray(w).astype(dt).reshape(2, P, C).transpose(1, 0, 2)
            .reshape(P, 2 * C))

    wa8 = np.concatenate(
        [wtile(inputs["xlin_w1"], ml_dtypes.float8_e4m3),
         wtile(inputs["xlin_w2"], ml_dtypes.float8_e4m3)], axis=1)
    onesb2 = np.concatenate(
        [np.ones((1, P), np.float32),
         np.asarray(inputs["xlin_b2"], np.float32).reshape(1, C)],
        axis=1).astype(ml_dtypes.float8_e4m3)
    wc = np.concatenate(
        [wtile(inputs[n], ml_dtypes.bfloat16)
         for n in ("xcn_w1", "xcn_w2", "xij_w", "lin_w1")], axis=1)

    def btile(b):
        return np.ascontiguousarray(
            np.asarray(b, dtype=np.float32).reshape(2, P).T)

    fpk = np.concatenate(
        [btile(inputs[n]) for n in
         ("xlin_b1", "xcn_b1", "xcn_b2", "xij_b", "lin_b1")] +
        [np.full((P, 1), np.asarray(inputs["beta"]).reshape(-1)[0],
                 dtype=np.float32),
         np.full((P, 1), np.asarray(inputs["lin_b2"]).reshape(-1)[0],
                 dtype=np.float32)],
        axis=1)

    common = {
        "x": x.astype(ml_dtypes.bfloat16),
        "xa8": xa8,
        "xr8t": xr8t,
        "adjs": adjs,
        "wa8": wa8,
        "onesb2": onesb2,
        "wc": wc,
        "fpk": fpk,
        "lin_w2": np.ascontiguousarray(inputs["lin_w2"]).astype(
            ml_dtypes.bfloat16),
    }

    in_maps = []
    for c in range(NCORES):
        m = dict(common)
        m["idx"] = np.ascontiguousarray(tar[:, c * EL:(c + 1) * EL])
        in_maps.append(m)

    res = run_bass_kernel_spmd(
        nc, in_maps, core_ids=list(range(NCORES)), trace=TRACE
    )
    global LAST_RESULT
    LAST_RESULT = res
    out = np.concatenate(
        [res.results[c]["out"].reshape(EL, 1) for c in range(NCORES)], axis=0
    )
    return out.astype(np.float32)
